# revision 77
# speedup vs baseline: 1.5215x; 1.0006x over previous
"""Trainium2 Bass kernel for nn_DiffusionOrderingNetwork (3-layer GAT, N=50000,
E=800000, softmax over nodes), SPMD across 8 NeuronCores.

Self-contained: host-side index/layout prep + Bass/Tile program + runner.
"""
import sys
sys.path.insert(0, '/opt/trn_rl_repo')
import numpy as np
import ml_dtypes
from contextlib import ExitStack

# ======================= host prep =======================
import numpy as _np

N = 50000
E = 800000
H = 6
C1 = 6
HID = 36
D = 64
NT = 17
NEG = 0.2
NCORES = 8
EPT = 128          # edges per tile
SPT = 8            # node slots per tile
KSUP = 32          # tiles per super-block (layers 1/2) -> 256 psum cols
KSUP3 = 16         # tiles per super-block (layer 3)   -> 128 psum cols


def _fold_ws(W, a):
    # ws[d, h] = sum_c W[d, h*C+c] * a[h, c]
    h, c = a.shape
    return np.einsum('dhc,hc->dh', W.reshape(W.shape[0], h, c), a).astype(np.float32)


def host_prep(x, edge_index, emb, w1, as1, ad1, b1, r1,
              w2, as2, ad2, b2, r2, w3, as3, ad3, b3, r3):
    x = np.asarray(x).astype(np.int64)
    ei = np.asarray(edge_index).astype(np.int64)
    N = x.shape[0]
    NT = emb.shape[0]
    D = emb.shape[1]
    for b in (b1, b2, b3):
        assert np.abs(np.asarray(b)).max() == 0.0, "nonzero bias breaks pad-column math"

    # --- edges sorted by dst; self-loops handled analytically on-device ---
    src = ei[0].copy()
    dst = ei[1].copy()
    order = np.argsort(dst, kind='stable')
    srcs = src[order]
    dsts = dst[order]
    ET = srcs.shape[0]
    deg = np.bincount(dst, minlength=N).astype(np.int64)
    assert deg.max() <= EPT, deg.max()
    node_ptr = np.concatenate([[0], np.cumsum(deg)])  # edge range per node

    # --- shard nodes into NCORES contiguous ranges with ~equal edges ---
    cum = np.cumsum(deg)
    bnds = [0]
    for k in range(1, NCORES):
        bnds.append(int(np.searchsorted(cum, ET * k / NCORES)))
    bnds.append(N)

    # --- per-core straddle tiling: a node's edges may split across two
    # adjacent tiles (never across a 16-tile super boundary); each tile has
    # at most SPT node starts; tiles fill to exactly EPT edges ---
    SB = 16
    core_tiles = []  # per core: list of tiles; tile = list of (node, e0, e1)
    for k in range(NCORES):
        nb, ne = bnds[k], bnds[k + 1]
        tiles = [[]]
        fill, starts = 0, 0
        for n in range(nb, ne):
            d = int(deg[n])
            e0 = int(node_ptr[n])
            if starts == SPT:
                tiles.append([])
                fill, starts = 0, 0
            starts += 1
            if d == 0:
                tiles[-1].append((n, e0, e0))
                continue
            rem = d
            while rem > 0:
                space = EPT - fill
                if space == 0:
                    tiles.append([])
                    fill, starts = 0, 1
                    space = EPT
                take = min(rem, space)
                if take < rem and (len(tiles) % SB) == 0:
                    # straddle would cross a super boundary: start fresh tile
                    tiles.append([])
                    fill, starts = 0, 1
                    take = min(rem, EPT)
                tiles[-1].append((n, e0, e0 + take))
                fill += take
                e0 += take
                rem -= take
        core_tiles.append(tiles)

    lcm = np.lcm(KSUP, np.lcm(KSUP3, 512 // SPT))  # tiles multiple for chunking
    TMAX = max(len(t) for t in core_tiles)
    TMAX = int(-(-TMAX // lcm) * lcm)
    SLOTMAX = TMAX * SPT
    NMAXOUT = max(bnds[k + 1] - bnds[k] for k in range(NCORES))
    NMAXOUT = int(-(-NMAXOUT // 128) * 128)

    # --- global slot-gid map (0 = poison row, then core-major slots);
    # a straddled node's slot lives in the tile of its FIRST edge segment ---
    nodeslot = np.zeros(N, dtype=np.int64)
    node_core = np.zeros(N, dtype=np.int64)
    core_slotof = [dict() for _ in range(NCORES)]  # node -> (tile, slot)
    for k in range(NCORES):
        slotof = core_slotof[k]
        for t, segs in enumerate(core_tiles[k]):
            nstart = 0
            for (n, e0, e1) in segs:
                if n not in slotof:
                    slotof[n] = (t, nstart)
                    nstart += 1
        for n, (t, s) in slotof.items():
            nodeslot[n] = 1 + k * SLOTMAX + t * SPT + s
            node_core[n] = k

    # --- per-core device input arrays ---
    # OHW/OHTW carry a 2-tile slot window per tile: cols/rows [0:SPT) are the
    # PREVIOUS tile's slots (for straddled nodes), [SPT:2*SPT) this tile's.
    W2 = 2 * SPT
    per_core = []
    for k in range(NCORES):
        tiles = core_tiles[k]
        nb, ne = bnds[k], bnds[k + 1]
        nreal = ne - nb
        slotof = core_slotof[k]

        srcgidT = np.zeros((EPT, TMAX), dtype=np.int32)
        OHW = np.zeros((EPT, TMAX * W2), dtype=np.float32)
        OHTW = np.zeros((W2, TMAX * EPT), dtype=np.float32)
        oh17 = np.zeros((NT, SLOTMAX), dtype=np.float32)
        n102 = np.zeros((H * NT, SLOTMAX), dtype=np.float32)
        outrowT = np.zeros((128, SLOTMAX // 128), dtype=np.int32) + 10**7
        slot_node = np.full(SLOTMAX, -1, dtype=np.int64)

        for t, segs in enumerate(tiles):
            row = 0
            for (n, e0, e1) in segs:
                ts, ss = slotof[n]
                assert ts in (t - 1, t)
                wcol = ss + (SPT if ts == t else 0)
                nseg = e1 - e0
                rows = row + np.arange(nseg)
                srcgidT[rows, t] = nodeslot[srcs[e0:e1]]
                OHW[rows, t * W2 + wcol] = 1.0
                OHTW[wcol, t * EPT + rows] = 1.0
                np.add.at(n102, (x[srcs[e0:e1]],
                                 np.full(nseg, ts * SPT + ss)), 1.0)
                row += nseg
                if ts == t:
                    sl = t * SPT + ss
                    oh17[x[n], sl] = 1.0
                    outrowT[sl % 128, sl // 128] = n - nb
                    slot_node[sl] = n - nb
            assert row <= EPT
        # nodes whose slot-tile had no edge segment in that tile (zero-deg or
        # straddle-only) are covered above since slots come from first segs.
        # self-loop counts for the layer-0 histogram:
        own = np.arange(nb, ne)
        sl_own = (nodeslot[own] - 1 - k * SLOTMAX).astype(np.int64)
        np.add.at(n102, (x[own], sl_own), 1.0)
        n102 = np.tile(n102[:NT], (H, 1))

        npadvec = np.full((D, 1), SLOTMAX - nreal, dtype=np.float32)
        per_core.append(dict(
            srcgidT=srcgidT, OH=OHW, OHT=OHTW, oh17=oh17, n102=n102,
            outrowT=outrowT, npadvec=npadvec, nreal=nreal, nb=nb, ne=ne,
            slot_node=slot_node,
        ))

    # --- folded weights (shared across cores) ---
    f32 = np.float32
    Wcat1 = np.concatenate([_fold_ws(w1, as1), w1.astype(f32), _fold_ws(w1, ad1)], axis=1)
    Wcat2 = np.concatenate([_fold_ws(w2, as2), w2.astype(f32), _fold_ws(w2, ad2)], axis=1)
    # layer 3: records carry xin itself (identity block); xs3 scores fold w3/as3
    Wcat3 = np.concatenate([_fold_ws(w3, as3), np.eye(HID, dtype=f32), _fold_ws(w3, ad3)], axis=1)
    # W3stack[h*HID+c, o] = w3[c, h*D+o] / H   (mean over heads folded in)
    W3stack = (w3.reshape(HID, H, D).transpose(1, 0, 2).reshape(H * HID, D) / H).astype(f32)
    REP2 = np.zeros((H, HID), dtype=f32)
    REP2[np.arange(HID) // C1, np.arange(HID)] = 1.0
    d3 = np.arange(H * HID)
    REP3A = np.zeros((H, 128), dtype=f32)
    REP3A[d3[:128] // HID, np.arange(128)] = 1.0
    REP3B = np.zeros((H, H * HID - 128), dtype=f32)
    REP3B[d3[128:] // HID, np.arange(H * HID - 128)] = 1.0
    zrow = np.zeros((SPT, 48), dtype=f32)
    # head selector for the (h, t)-major layer-0 histogram rows
    SELH2 = np.zeros((H * NT, H), dtype=f32)
    SELH2[np.arange(H * NT), np.arange(H * NT) // NT] = 1.0
    # head-replication of hidden features for the layer-3 self-term
    T6A = np.zeros((HID, 128), dtype=f32)
    T6A[np.arange(128) % HID, np.arange(128)] = 1.0
    T6B = np.zeros((HID, H * HID - 128), dtype=f32)
    T6B[np.arange(128, H * HID) % HID, np.arange(H * HID - 128)] = 1.0

    shared = dict(
        emb=emb.astype(f32), embT=emb.astype(f32).T.copy(),
        Wcat1=Wcat1, Wcat2=Wcat2, Wcat3=Wcat3,
        W3stackA=W3stack[:128].copy(), W3stackB=W3stack[128:].copy(),
        r1=r1.astype(f32), r2=r2.astype(f32), r3=r3.astype(f32),
        b1=b1.astype(f32).reshape(-1, 1), b2=b2.astype(f32).reshape(-1, 1),
        b3=b3.astype(f32).reshape(-1, 1),
        REP2=REP2, REP3A=REP3A, REP3B=REP3B, zrow=zrow, SELH2=SELH2,
        T6A=T6A, T6B=T6B,
    )
    meta = dict(TMAX=TMAX, SLOTMAX=SLOTMAX, NMAXOUT=NMAXOUT,
                bnds=bnds, nreal=[pc['nreal'] for pc in per_core],
                slot_node=[pc['slot_node'] for pc in per_core])
    return per_core, shared, meta


def numpy_reference(x, edge_index, emb, w1, as1, ad1, b1, r1,
                    w2, as2, ad2, b2, r2, w3, as3, ad3, b3, r3):
    """Plain numpy port of reference.py for quick host validation."""
    def gat(xf, src, dst, W, a_s, a_d, b, r, concat):
        n = xf.shape[0]
        h, c = a_s.shape
        xs = (xf @ W).reshape(n, h, c)
        a_src = (xs * a_s).sum(-1)
        a_dst = (xs * a_d).sum(-1)
        e = a_src[src] + a_dst[dst]
        e = np.where(e > 0, e, NEG * e)
        m = np.full((n, h), -np.inf)
        np.maximum.at(m, dst, e)
        m = np.where(np.isfinite(m), m, 0.0)
        ex = np.exp(e - m[dst])
        s = np.zeros((n, h))
        np.add.at(s, dst, ex)
        alpha = ex / (s[dst] + 1e-16)
        out = np.zeros((n, h, c))
        np.add.at(out, dst, xs[src] * alpha[:, :, None])
        out = out.reshape(n, h * c) if concat else out.mean(1)
        return out + xf @ r + b

    hf = emb[np.asarray(x).astype(np.int64)]
    loops = np.arange(x.shape[0])
    src = np.concatenate([edge_index[0], loops])
    dst = np.concatenate([edge_index[1], loops])
    hf = np.maximum(gat(hf, src, dst, w1, as1, ad1, b1, r1, True), 0)
    hf = np.maximum(gat(hf, src, dst, w2, as2, ad2, b2, r2, True), 0)
    hf = gat(hf, src, dst, w3, as3, ad3, b3, r3, False)
    hf = hf - hf.max(0, keepdims=True)
    e = np.exp(hf)
    return (e / e.sum(0, keepdims=True)).astype(np.float32)


# ======================= device program =======================

import concourse.bass as bass
import concourse.tile as tile
from concourse import bacc, mybir
from concourse.masks import make_identity
from concourse.tile import add_dep_helper

F32 = mybir.dt.float32
I32 = mybir.dt.int32
BF16 = mybir.dt.bfloat16

H = 6
EPT = 128
SPT = 8
KSUP = 32      # tiles per super for layers 1/2 (256 psum cols)
KSUP3 = 16     # tiles per super for layer 3  (128 psum cols)


def build_program(TMAX, SLOTMAX, NMAXOUT, D, HID, NT, n_cores=8, edge_dt=BF16,
                  debug_dump=False):
    RW = 48                      # record row: asrc(6) | xs(HID=36) | adst(6)
    NCH128 = SLOTMAX // 128
    NCH512 = SLOTMAX // 512
    TROWS = 1 + n_cores * SLOTMAX
    V216 = H * HID               # 216
    VA = 128                     # layer-3 agg split A (dims 0:128)
    VB = V216 - 128              # 88
    cores = list(range(n_cores))

    nc = bacc.Bacc("TRN2", target_bir_lowering=False, debug=False,
                   num_devices=n_cores)

    def din(name, shape, dt=F32):
        return nc.dram_tensor(name, list(shape), dt, kind="ExternalInput")

    W2 = 2 * SPT
    srcg_d = din("srcgidT", [EPT, TMAX], I32)
    oh_d = din("OH", [EPT, TMAX * W2], edge_dt)
    ohtw_d = din("OHTW", [W2, TMAX * EPT], edge_dt)
    oh17_d = din("oh17", [NT, SLOTMAX], edge_dt)
    n102_d = din("n102", [H * NT, SLOTMAX], edge_dt)
    selh2_d = din("SELH2", [H * NT, H], edge_dt)
    t6a_d = din("T6A", [HID, VA], edge_dt)
    t6b_d = din("T6B", [HID, VB], edge_dt)
    outr_d = din("outrowT", [128, NCH128], I32)
    npad_d = din("npadvec", [D, 1])
    embt_d = din("embT", [D, NT], edge_dt)
    wcat_d = [din("Wcat1", [D, RW], edge_dt), din("Wcat2", [HID, RW], edge_dt),
              din("Wcat3", [HID, RW], edge_dt)]
    w3a_d = din("W3stackA", [VA, D], edge_dt)
    w3b_d = din("W3stackB", [VB, D], edge_dt)
    r_d = [din("r1", [D, HID], edge_dt), din("r2", [HID, HID], edge_dt),
           din("r3", [HID, D], edge_dt)]
    b_d = [din("b1", [HID, 1]), din("b2", [HID, 1]), din("b3", [D, 1])]
    rep2_d = din("REP2", [H, HID])
    rep3a_d = din("REP3A", [H, VA])
    rep3b_d = din("REP3B", [H, VB])
    zrow_d = din("zrow", [SPT, RW], edge_dt)
    out_d = nc.dram_tensor("out", [SLOTMAX, D], F32, kind="ExternalOutput")
    if debug_dump:
        dbg1_d = nc.dram_tensor("dbg1", [HID, SLOTMAX], edge_dt, kind="ExternalOutput")
        dbg2_d = nc.dram_tensor("dbg2", [HID, SLOTMAX], edge_dt, kind="ExternalOutput")
        dbg3_d = nc.dram_tensor("dbg3", [D, SLOTMAX], F32, kind="ExternalOutput")

    ag_in = nc.dram_tensor("ag_in", [SLOTMAX, RW], edge_dt)
    table = nc.dram_tensor("table", [TROWS, RW], edge_dt)
    adstL = nc.dram_tensor("adstL", [SPT + SLOTMAX, H], edge_dt)
    cca_i = nc.dram_tensor("cca_i", [D, 1], F32)
    cca_o = nc.dram_tensor("cca_o", [D, 1], F32)
    ccs_i = nc.dram_tensor("ccs_i", [D, 1], F32)
    ccs_o = nc.dram_tensor("ccs_o", [D, 1], F32)

    with ExitStack() as ctx:
        tc = ctx.enter_context(tile.TileContext(nc))
        res = ctx.enter_context(tc.tile_pool(name="res", bufs=1))
        cst = ctx.enter_context(tc.tile_pool(name="cst", bufs=1))
        aux = ctx.enter_context(tc.tile_pool(name="aux", bufs=2, space="PSUM"))
        p1p = ctx.enter_context(tc.tile_pool(name="p1p", bufs=3))

        def load(pool, src, shape, dt=F32, tag=None):
            t = pool.tile(list(shape), dt, tag=tag)
            nc.sync.dma_start(out=t[:], in_=src[:])
            return t

        srcg = load(res, srcg_d, [EPT, TMAX], I32, tag="srcg")
        oht_sb = load(res, oh_d, [EPT, TMAX * W2], edge_dt, tag="oht")
        outr = load(cst, outr_d, [128, NCH128], I32, tag="outr")
        t6a_sb = load(cst, t6a_d, [HID, VA], edge_dt, tag="t6a")
        t6b_sb = load(cst, t6b_d, [HID, VB], edge_dt, tag="t6b")
        npad_sb = load(cst, npad_d, [D, 1], tag="npad")
        embt_sb = load(cst, embt_d, [D, NT], edge_dt, tag="embt")
        selh2_sb = load(cst, selh2_d, [H * NT, H], edge_dt, tag="selh2")
        wcat_sb = [load(cst, wcat_d[0], [D, RW], edge_dt, tag="wc1"),
                   load(cst, wcat_d[1], [HID, RW], edge_dt, tag="wc2"),
                   load(cst, wcat_d[2], [HID, RW], edge_dt, tag="wc3")]
        w3a_sb = load(cst, w3a_d, [VA, D], edge_dt, tag="w3a")
        w3b_sb = load(cst, w3b_d, [VB, D], edge_dt, tag="w3b")
        r_sb = [load(cst, r_d[0], [D, HID], edge_dt, tag="r1"),
                load(cst, r_d[1], [HID, HID], edge_dt, tag="r2"),
                load(cst, r_d[2], [HID, D], edge_dt, tag="r3")]
        b_sb = [load(cst, b_d[0], [HID, 1], tag="b1"),
                load(cst, b_d[1], [HID, 1], tag="b2"),
                load(cst, b_d[2], [D, 1], tag="b3")]
        rep2_sb = load(cst, rep2_d, [H, HID], tag="rep2")
        rep3a_sb = load(cst, rep3a_d, [H, VA], tag="rep3a")
        rep3b_sb = load(cst, rep3b_d, [H, VB], tag="rep3b")
        idn = cst.tile([64, 64], F32, tag="idn")
        make_identity(nc, idn[:])
        nc.sync.dma_start(out=table[0:1, :], in_=zrow_d[0:1, :])
        adz = nc.sync.dma_start(out=adstL[0:SPT, :], in_=zrow_d[:, 0:H])

        # ---- t17 = per-type layer-1 records [NT, RW]; er1 = emb @ r1 ----
        V102 = H * NT
        t17_sb = cst.tile([NT, RW], edge_dt, tag="t17")
        t17f_sb = cst.tile([NT, RW], F32, tag="t17f")
        er1_sb = cst.tile([NT, HID], edge_dt, tag="er1")
        with tc.tile_pool(name="p17", bufs=1, space="PSUM") as p17:
            ps = p17.tile([NT, RW], F32, space="PSUM", tag="ps")
            nc.tensor.matmul(out=ps[:], lhsT=embt_sb[:], rhs=wcat_sb[0][:],
                             start=True, stop=True)
            nc.vector.tensor_copy(t17_sb[:], ps[:])
            nc.vector.tensor_copy(t17f_sb[:], ps[:])
            pse = p17.tile([NT, HID], F32, space="PSUM", tag="pse")
            nc.tensor.matmul(out=pse[:], lhsT=embt_sb[:], rhs=r_sb[0][:],
                             start=True, stop=True)
            nc.vector.tensor_copy(er1_sb[:], pse[:])

        # ---- layer-0 histogram operands derived from t17 ----
        # rows are (h, t)-major: row h*NT+t
        L_sb = cst.tile([NT, V102], edge_dt, tag="Lsb")       # ad expander
        at102 = cst.tile([V102, 1], F32, tag="at102")         # a_src per (h,t)
        w17t = cst.tile([V102, HID], edge_dt, tag="w17t")     # xs selector
        nc.vector.memset(w17t[:], 0.0)
        for h in range(H):
            nc.vector.tensor_copy(
                L_sb[:, h * NT:(h + 1) * NT],
                t17_sb[:, RW - H + h:RW - H + h + 1].to_broadcast([NT, NT]))
            # partition-shifted moves must go through DMA, not DVE
            nc.sync.dma_start(out=at102[h * NT:(h + 1) * NT, :],
                              in_=t17f_sb[:, h:h + 1])
            c0 = H + h * (HID // H)
            nc.sync.dma_start(
                out=w17t[h * NT:(h + 1) * NT,
                         h * (HID // H):(h + 1) * (HID // H)],
                in_=t17_sb[:, c0:c0 + HID // H])

        lsum = cst.tile([D, 1], F32, tag="lsum")
        lsum2 = cst.tile([D, 1], F32, tag="lsum2")
        # summed (asrc + adst) weight columns for the self-loop terms
        wsum1 = cst.tile([HID, H], edge_dt, tag="wsum1")
        wsum2 = cst.tile([HID, H], edge_dt, tag="wsum2")
        wsum_sb = [wsum1, wsum2]
        for i in (0, 1):
            nc.vector.tensor_tensor(out=wsum_sb[i][:],
                                    in0=wcat_sb[i + 1][:, 0:H],
                                    in1=wcat_sb[i + 1][:, RW - H:RW],
                                    op=mybir.AluOpType.add)

        hT1 = res.tile([HID, SLOTMAX], edge_dt, tag="h36a")
        hT2 = res.tile([HID, SLOTMAX], edge_dt, tag="h36b")
        out3T = res.tile([D, SLOTMAX], F32, tag="h64")
        agg3A = res.tile([VA, SLOTMAX], edge_dt, tag="agg3A")
        agg3B = res.tile([VB, SLOTMAX], edge_dt, tag="agg3B")
        h6a_sb = res.tile([VA, SLOTMAX], edge_dt, tag="h6a")
        h6b_sb = res.tile([VB, SLOTMAX], edge_dt, tag="h6b")

        # pipelined record-phase: emit one 128-col record chunk for layer l
        hmap = {1: hT1, 2: hT2}
        adw_map = {1: [], 2: []}
        agst = {'cc': None}

        def emit_p1(l, c4):
            # one 512-slot group: 4 record matmuls, one sb tile, 2 DMAs
            sb4 = p1p.tile([128, 4 * RW], edge_dt, tag="sb")
            for j in range(4):
                c = 4 * c4 + j
                ps1 = aux.tile([128, 512], F32, space="PSUM", tag="aux")
                nc.tensor.matmul(out=ps1[:, 0:RW],
                                 lhsT=hmap[l][:, c * 128:(c + 1) * 128],
                                 rhs=wcat_sb[l][:], start=True, stop=True)
                nc.scalar.copy(out=sb4[:, j * RW:(j + 1) * RW],
                               in_=ps1[:, 0:RW])
            sb3 = sb4[:].rearrange("p (j d) -> p j d", d=RW)
            wdma = nc.sync.dma_start(
                out=ag_in[c4 * 512:(c4 + 1) * 512,
                          :].rearrange("(j p) d -> p j d", p=128),
                in_=sb3)
            if agst['cc'] is not None:
                for _cc in agst['cc']:
                    add_dep_helper(wdma.ins, _cc.ins,
                                   reason="ag_in WAR vs previous AllGather")
            adw = nc.sync.dma_start(
                out=adstL[SPT + c4 * 512:SPT + (c4 + 1) * 512,
                          :].rearrange("(j p) d -> p j d", p=128),
                in_=sb3[:, :, RW - H:RW])
            adw_map[l].append(adw)

        # ---- layer 0: per-slot type-histogram GAT (no per-edge work) ----
        with tc.tile_pool(name="l0", bufs=3) as p0, \
             tc.tile_pool(name="l0in", bufs=1) as pin, \
             tc.tile_pool(name="l0a", bufs=2, space="PSUM") as pA, \
             tc.tile_pool(name="l0b", bufs=1, space="PSUM") as pB, \
             tc.tile_pool(name="l0c", bufs=1, space="PSUM") as pC:
            oh17_sb = pin.tile([NT, SLOTMAX], edge_dt, tag="oh17s")
            n102_sb = pin.tile([V102, SLOTMAX], edge_dt, tag="n102s")
            hsm = SLOTMAX // 2
            nc.sync.dma_start(out=oh17_sb[:, 0:hsm], in_=oh17_d[:, 0:hsm])
            nc.sync.dma_start(out=n102_sb[:, 0:hsm], in_=n102_d[:, 0:hsm])
            nc.sync.dma_start(out=oh17_sb[:, hsm:], in_=oh17_d[:, hsm:])
            nc.sync.dma_start(out=n102_sb[:, hsm:], in_=n102_d[:, hsm:])
            for c in range(NCH512):
                csl = slice(c * 512, (c + 1) * 512)
                ohc = oh17_sb[:, csl]
                n102c = n102_sb[:, csl]
                ps102 = pA.tile([V102, 512], F32, space="PSUM", tag="ps102")
                nc.tensor.matmul(out=ps102[:], lhsT=L_sb[:], rhs=ohc,
                                 start=True, stop=True)
                esc = p0.tile([V102, 512], F32, tag="esc0")
                nc.vector.tensor_scalar_add(out=esc[:], in0=ps102[:],
                                            scalar1=at102[:])
                nc.vector.scalar_tensor_tensor(
                    out=esc[:], in0=esc[:], scalar=0.2, in1=esc[:],
                    op0=mybir.AluOpType.mult, op1=mybir.AluOpType.max)
                nc.scalar.activation(out=esc[:], in_=esc[:],
                                     func=mybir.ActivationFunctionType.Exp)
                nE = p0.tile([V102, 512], edge_dt, tag="nE")
                nc.gpsimd.tensor_tensor(out=nE[:], in0=esc[:], in1=n102c,
                                        op=mybir.AluOpType.mult)
                psD = pC.tile([H, 512], F32, space="PSUM", tag="psD")
                nc.tensor.matmul(out=psD[:], lhsT=selh2_sb[:], rhs=nE[:],
                                 start=True, stop=True)
                psN = pB.tile([HID, 512], F32, space="PSUM", tag="psN")
                nc.tensor.matmul(out=psN[:], lhsT=w17t[:], rhs=nE[:],
                                 start=True, stop=True)
                psR = pB.tile([HID, 512], F32, space="PSUM", tag="psR")
                nc.tensor.matmul(out=psR[:], lhsT=er1_sb[:], rhs=ohc,
                                 start=True, stop=True)
                rs = p0.tile([H, 512], F32, tag="rs0")
                nc.vector.tensor_scalar_add(out=rs[:], in0=psD[:],
                                            scalar1=1e-16)
                nc.vector.reciprocal(out=rs[:], in_=rs[:])
                ps2 = pC.tile([HID, 512], F32, space="PSUM", tag="ps20")
                nc.tensor.matmul(out=ps2[:], lhsT=rep2_sb[:], rhs=rs[:],
                                 start=True, stop=True)
                rr = p0.tile([HID, 512], F32, tag="rr0")
                nc.scalar.copy(out=rr[:], in_=ps2[:])
                nc.vector.tensor_tensor(out=hT1[:, csl], in0=psN[:], in1=rr[:],
                                        op=mybir.AluOpType.mult)
                nc.vector.tensor_tensor(out=hT1[:, csl], in0=hT1[:, csl],
                                        in1=psR[:], op=mybir.AluOpType.add)
                nc.scalar.activation(out=hT1[:, csl], in_=hT1[:, csl],
                                     func=mybir.ActivationFunctionType.Relu,
                                     bias=b_sb[0][:])
                emit_p1(1, c)
        if debug_dump:
            nc.sync.dma_start(out=dbg1_d[:], in_=hT1[:])

        hins = [None, hT1, hT2]
        houts = [None, hT2, None]
        prev_cc = None
        prev_readers = []

        for l in (1, 2):
            hin = hins[l]
            adst_writes = adw_map[l]

            # ---- P2: all-gather the record table (written by pipelined P1) ----
            if n_cores == 1:
                # model the collective as 8 parallel chunk copies
                ccs_l = []
                nch8 = SLOTMAX // 8
                for i8 in range(8):
                    cci = nc.sync.dma_start(
                        out=table[1 + i8 * nch8:1 + (i8 + 1) * nch8, :],
                        in_=ag_in[i8 * nch8:(i8 + 1) * nch8, :])
                    ccs_l.append(cci)
            else:
                ccs_l = [nc.gpsimd.collective_compute(
                    "AllGather", mybir.AluOpType.bypass,
                    replica_groups=[cores],
                    ins=[ag_in[:]], outs=[table[1:, :]],
                )]
            for cc in ccs_l:
                for rd in prev_readers:
                    add_dep_helper(cc.ins, rd.ins,
                                   reason="table WAR vs previous layer gathers")
            prev_cc = ccs_l
            agst['cc'] = ccs_l
            prev_readers = []

            # ---- P3: edge phase (scatter windows are 2 tiles wide: a node
            # may straddle into the next tile; psum accumulates) ----
            ks = KSUP if l < 2 else KSUP3
            nsup = TMAX // ks
            lw = RW - H if l < 2 else H + V216   # scatter lhsT width: 42 / 222
            cols = ks * SPT                      # real psum cols per super
            pcols = cols + SPT                   # + leading ghost window
            with tc.tile_pool(name=f"ed{l}", bufs=3) as wp, \
                 tc.tile_pool(name=f"edp{l}", bufs=1, space="PSUM") as pp, \
                 tc.tile_pool(name=f"eds{l}", bufs=1, space="PSUM") as pps, \
                 tc.tile_pool(name=f"tmp{l}", bufs=1, space="PSUM") as tpp, \
                 tc.tile_pool(name=f"rcp{l}", bufs=1, space="PSUM") as tpr, \
                 tc.tile_pool(name=f"adp{l}", bufs=1, space="PSUM") as adp:
                for g in range(nsup):
                    t0 = g * ks
                    csl = slice(g * cols, (g + 1) * cols)
                    Rg = wp.tile([EPT, ks * RW], edge_dt, tag="Rg")
                    for k in range(ks):
                        gi = nc.gpsimd.indirect_dma_start(
                            out=Rg[:, k * RW:(k + 1) * RW],
                            out_offset=None, in_=table[:],
                            in_offset=bass.IndirectOffsetOnAxis(
                                ap=srcg[:, t0 + k:t0 + k + 1], axis=0))
                        for _cc in prev_cc:
                            add_dep_helper(gi.ins, _cc.ins,
                                           reason="gather RAW AllGather")
                        prev_readers.append(gi)
                    # a_dst expansion operands: 16-row window = prev|own slots
                    ohts = wp.tile([W2, ks * EPT], edge_dt, tag="ohts")
                    nc.sync.dma_start(out=ohts[:],
                                      in_=ohtw_d[:, t0 * EPT:(t0 + ks) * EPT])
                    adsw = wp.tile([W2, ks * H], edge_dt, tag="adsw")
                    adr0 = nc.sync.dma_start(
                        out=adsw[0:SPT, :].rearrange("s (k e) -> s k e", e=H),
                        in_=adstL[t0 * SPT:(t0 + ks) * SPT, :].rearrange(
                            "(k s) e -> s k e", s=SPT))
                    adr1 = nc.sync.dma_start(
                        out=adsw[SPT:W2, :].rearrange("s (k e) -> s k e", e=H),
                        in_=adstL[(t0 + 1) * SPT:(t0 + ks + 1) * SPT,
                                  :].rearrange("(k s) e -> s k e", s=SPT))
                    for c in range(max(0, (t0 * SPT - SPT)) // 512,
                                   ((t0 + ks) * SPT + 511) // 512):
                        add_dep_helper(adr0.ins, adst_writes[c].ins,
                                       reason="ads RAW adstL chunk write")
                        add_dep_helper(adr1.ins, adst_writes[c].ins,
                                       reason="ads RAW adstL chunk write")
                    if g == 0:
                        add_dep_helper(adr0.ins, adz.ins,
                                       reason="ads RAW adstL zero rows")
                    psAD = adp.tile([EPT, ks * H], F32, space="PSUM", tag="psAD")
                    for k in range(ks):
                        nc.tensor.matmul(
                            out=psAD[:, k * H:(k + 1) * H],
                            lhsT=ohts[:, k * EPT:(k + 1) * EPT],
                            rhs=adsw[:, k * H:(k + 1) * H],
                            start=True, stop=True)
                    R3 = Rg[:].rearrange("p (k e) -> p k e", e=RW)
                    esc = wp.tile([EPT, ks * H], F32, tag="esc")
                    nc.vector.tensor_tensor(
                        out=esc[:], in0=R3[:, :, 0:H],
                        in1=psAD[:], op=mybir.AluOpType.add)
                    nc.vector.scalar_tensor_tensor(
                        out=esc[:], in0=esc[:], scalar=0.2, in1=esc[:],
                        op0=mybir.AluOpType.mult, op1=mybir.AluOpType.max)
                    RHS = wp.tile([EPT, ks * lw], edge_dt, tag="RHS")
                    S3 = RHS[:].rearrange("p (k e) -> p k e", e=lw)
                    nc.scalar.activation(
                        out=S3[:, :, 0:H],
                        in_=esc[:].rearrange("p (k h) -> p k h", h=H),
                        func=mybir.ActivationFunctionType.Exp)
                    ex_rep = S3[:, :, 0:H][:, :, :, None].to_broadcast(
                        [EPT, ks, H, lw // H - 1])
                    if l < 2:
                        xs_in = R3[:, :, H:RW - H].rearrange(
                            "p k (h c) -> p k h c", h=H)
                    else:
                        xs_in = R3[:, :, H:RW - H][:, :, None, :].to_broadcast(
                            [EPT, ks, H, HID])
                    nc.vector.tensor_tensor(
                        out=S3[:, :, H:lw].rearrange("p k (h c) -> p k h c", h=H),
                        in0=xs_in, in1=ex_rep, op=mybir.AluOpType.mult)
                    # self-loop term: esc_self = hin.T @ (W_asrc + W_adst)
                    psRec = tpr.tile([H, cols], F32, space="PSUM",
                                     tag="psRec")
                    nc.tensor.matmul(out=psRec[:], lhsT=wsum_sb[l - 1][:],
                                     rhs=hin[:, csl], start=True, stop=True)
                    e3 = wp.tile([H, cols], F32, tag="e3")
                    nc.scalar.copy(out=e3[:], in_=psRec[:])
                    nc.vector.scalar_tensor_tensor(
                        out=e3[:], in0=e3[:], scalar=0.2, in1=e3[:],
                        op0=mybir.AluOpType.mult, op1=mybir.AluOpType.max)
                    nc.scalar.activation(out=e3[:], in_=e3[:],
                                         func=mybir.ActivationFunctionType.Exp)
                    # scatter with 16-col windows, accumulating
                    psS = (pp if l < 2 else pps).tile([H, pcols], F32,
                                                      space="PSUM", tag="psS")
                    nc.vector.memset(psS[:], 0.0)
                    if l < 2:
                        psV = pp.tile([HID, pcols], F32, space="PSUM", tag="psV")
                        nc.vector.memset(psV[:], 0.0)
                    else:
                        psA = pp.tile([VA, pcols], F32, space="PSUM", tag="psA")
                        psB = pp.tile([VB, pcols], F32, space="PSUM", tag="psB")
                        nc.vector.memset(psA[:], 0.0)
                        nc.vector.memset(psB[:], 0.0)
                    for k in range(ks):
                        t = t0 + k
                        ohs = oht_sb[:, t * W2:(t + 1) * W2]
                        wsl = slice(k * SPT, k * SPT + W2)
                        lb = k * lw
                        nc.tensor.matmul(
                            out=psS[:, wsl],
                            lhsT=RHS[:, lb:lb + H], rhs=ohs,
                            start=False, stop=True)
                        if l < 2:
                            nc.tensor.matmul(
                                out=psV[:, wsl],
                                lhsT=RHS[:, lb + H:lb + lw], rhs=ohs,
                                start=False, stop=True)
                        else:
                            nc.tensor.matmul(
                                out=psA[:, wsl],
                                lhsT=RHS[:, lb + H:lb + H + VA], rhs=ohs,
                                start=False, stop=True)
                            nc.tensor.matmul(
                                out=psB[:, wsl],
                                lhsT=RHS[:, lb + H + VA:lb + lw], rhs=ohs,
                                start=False, stop=True)
                    rs = wp.tile([H, cols], F32, tag="rs")
                    nc.vector.scalar_tensor_tensor(
                        out=rs[:], in0=psS[:, SPT:], scalar=1e-16, in1=e3[:],
                        op0=mybir.AluOpType.add, op1=mybir.AluOpType.add)
                    nc.vector.reciprocal(out=rs[:], in_=rs[:])
                    z = wp.tile([H, cols], F32, tag="z")
                    nc.vector.tensor_tensor(out=z[:], in0=e3[:], in1=rs[:],
                                            op=mybir.AluOpType.mult)
                    if l < 2:
                        ps2 = tpp.tile([HID, cols], F32, space="PSUM",
                                       tag="ps2")
                        nc.tensor.matmul(out=ps2[:], lhsT=rep2_sb[:], rhs=rs[:],
                                         start=True, stop=True)
                        rr = wp.tile([HID, cols], F32, tag="rr")
                        nc.scalar.copy(out=rr[:], in_=ps2[:])
                        psZ = tpp.tile([HID, cols], F32, space="PSUM",
                                       tag="ps2")
                        nc.tensor.matmul(out=psZ[:], lhsT=rep2_sb[:], rhs=z[:],
                                         start=True, stop=True)
                        zz = wp.tile([HID, cols], F32, tag="zz")
                        nc.scalar.copy(out=zz[:], in_=psZ[:])
                        # self value xs_self = wcat[:, H:RW-H].T @ hin
                        psXS = tpp.tile([HID, cols], F32, space="PSUM",
                                        tag="ps2")
                        nc.tensor.matmul(out=psXS[:],
                                         lhsT=wcat_sb[l][:, H:RW - H],
                                         rhs=hin[:, csl], start=True, stop=True)
                        nc.vector.tensor_tensor(
                            out=zz[:], in0=zz[:], in1=psXS[:],
                            op=mybir.AluOpType.mult)
                        nc.vector.tensor_tensor(
                            out=rr[:], in0=psV[:, SPT:], in1=rr[:],
                            op=mybir.AluOpType.mult)
                        nc.vector.tensor_tensor(
                            out=houts[l][:, csl], in0=rr[:], in1=zz[:],
                            op=mybir.AluOpType.add)
                    else:
                        for rep_sb, psX, h6, agg, vx in (
                                (rep3a_sb, psA, h6a_sb, agg3A, VA),
                                (rep3b_sb, psB, h6b_sb, agg3B, VB)):
                            ps2X = tpp.tile([vx, cols], F32, space="PSUM",
                                            tag="ps2")
                            nc.tensor.matmul(out=ps2X[:], lhsT=rep_sb[:],
                                             rhs=rs[:], start=True, stop=True)
                            rrX = wp.tile([vx, cols], F32, tag="rrX")
                            nc.scalar.copy(out=rrX[:], in_=ps2X[:])
                            psZX = tpp.tile([vx, cols], F32, space="PSUM",
                                            tag="ps2")
                            nc.tensor.matmul(out=psZX[:], lhsT=rep_sb[:],
                                             rhs=z[:], start=True, stop=True)
                            zzX = wp.tile([vx, cols], F32, tag="zzX")
                            nc.scalar.copy(out=zzX[:], in_=psZX[:])
                            nc.vector.tensor_tensor(
                                out=zzX[:], in0=zzX[:], in1=h6[:, csl],
                                op=mybir.AluOpType.mult)
                            nc.vector.tensor_tensor(
                                out=rrX[:], in0=psX[:, SPT:], in1=rrX[:],
                                op=mybir.AluOpType.mult)
                            nc.vector.tensor_tensor(
                                out=agg[:, csl], in0=rrX[:], in1=zzX[:],
                                op=mybir.AluOpType.add)

                    # ---- pipelined finalize of completed 512-col chunks ----
                    if l == 1 and g % 2 == 1:
                        c5 = g // 2
                        csl5 = slice(c5 * 512, (c5 + 1) * 512)
                        psr = aux.tile([128, 512], F32, space="PSUM", tag="aux")
                        nc.tensor.matmul(out=psr[0:HID, :], lhsT=r_sb[1][:],
                                         rhs=hT1[:, csl5], start=True, stop=True)
                        nc.vector.tensor_tensor(out=hT2[:, csl5],
                                                in0=hT2[:, csl5],
                                                in1=psr[0:HID, :],
                                                op=mybir.AluOpType.add)
                        nc.scalar.activation(
                            out=hT2[:, csl5], in_=hT2[:, csl5],
                            func=mybir.ActivationFunctionType.Relu,
                            bias=b_sb[1][:])
                        emit_p1(2, c5)
                        psh = aux.tile([128, 512], F32, space="PSUM", tag="aux")
                        nc.tensor.matmul(out=psh[0:VA, :], lhsT=t6a_sb[:],
                                         rhs=hT2[:, csl5], start=True, stop=True)
                        nc.scalar.copy(out=h6a_sb[:, csl5], in_=psh[0:VA, :])
                        psh2 = aux.tile([128, 512], F32, space="PSUM", tag="aux")
                        nc.tensor.matmul(out=psh2[0:VB, :], lhsT=t6b_sb[:],
                                         rhs=hT2[:, csl5], start=True, stop=True)
                        nc.scalar.copy(out=h6b_sb[:, csl5], in_=psh2[0:VB, :])
                    if l == 2 and g % 4 == 3:
                        c5 = g // 4
                        csl5 = slice(c5 * 512, (c5 + 1) * 512)
                        ps3 = aux.tile([128, 512], F32, space="PSUM", tag="aux")
                        nc.tensor.matmul(out=ps3[0:D, :], lhsT=w3a_sb[:],
                                         rhs=agg3A[:, csl5],
                                         start=True, stop=False)
                        nc.tensor.matmul(out=ps3[0:D, :], lhsT=w3b_sb[:],
                                         rhs=agg3B[:, csl5],
                                         start=False, stop=False)
                        nc.tensor.matmul(out=ps3[0:D, :], lhsT=r_sb[2][:],
                                         rhs=hT2[:, csl5],
                                         start=False, stop=True)
                        nc.vector.tensor_scalar_add(out=out3T[:, csl5],
                                                    in0=ps3[0:D, :],
                                                    scalar1=b_sb[2][:])
                        # logits are O(1): exp + sum need no max-subtraction
                        nc.scalar.activation(
                            out=out3T[:, csl5], in_=out3T[:, csl5],
                            func=mybir.ActivationFunctionType.Exp)
                        lsc = wp.tile([D, 1], F32, tag="lsc")
                        nc.vector.tensor_reduce(out=lsc[:],
                                                in_=out3T[:, csl5],
                                                axis=mybir.AxisListType.X,
                                                op=mybir.AluOpType.add)
                        acc = lsum if c5 % 2 == 0 else lsum2
                        if c5 < 2:
                            nc.vector.tensor_copy(acc[:], lsc[:])
                        else:
                            nc.vector.tensor_tensor(out=acc[:], in0=acc[:],
                                                    in1=lsc[:],
                                                    op=mybir.AluOpType.add)

            if debug_dump and l == 1:
                nc.sync.dma_start(out=dbg2_d[:], in_=hT2[:])

        # ---- P5: softmax over nodes (global across cores) ----
        with tc.tile_pool(name="sm", bufs=2) as sp, \
             tc.tile_pool(name="smp", bufs=2, space="PSUM") as spp:
            nc.vector.tensor_tensor(out=lsum[:], in0=lsum[:], in1=lsum2[:],
                                    op=mybir.AluOpType.add)
            # pad slots each contribute exp(0) = 1; subtract their count
            nc.vector.tensor_sub(out=lsum[:], in0=lsum[:], in1=npad_sb[:])
            nc.sync.dma_start(out=ccs_i[:], in_=lsum[:])
            if n_cores == 1:
                cc2 = nc.sync.dma_start(out=ccs_o[:], in_=ccs_i[:])
            else:
                cc2 = nc.gpsimd.collective_compute(
                    "AllReduce", mybir.AluOpType.add, replica_groups=[cores],
                    ins=[ccs_i[:]], outs=[ccs_o[:]])
            gsum = sp.tile([D, 1], F32, tag="gsum")
            rb2 = nc.sync.dma_start(out=gsum[:], in_=ccs_o[:])
            add_dep_helper(rb2.ins, cc2.ins, reason="read AllReduce sum result")
            nc.vector.reciprocal(out=gsum[:], in_=gsum[:])
            # fold 1/gsum into a diagonal matmul that also transposes
            diagS = sp.tile([64, 64], F32, tag="diagS")
            nc.vector.tensor_scalar_mul(out=diagS[:], in0=idn[:],
                                        scalar1=gsum[:])
            if debug_dump:
                nc.vector.tensor_scalar_mul(out=out3T[:], in0=out3T[:],
                                            scalar1=gsum[:])
                nc.sync.dma_start(out=dbg3_d[:], in_=out3T[:])
            for c4 in range(NCH128 // 4):
                csl4 = slice(c4 * 512, (c4 + 1) * 512)
                ev4 = sp.tile([128, 4 * D], F32, tag="ev4")
                for j in range(4):
                    trp = spp.tile([128, D], F32, space="PSUM", tag="trp")
                    nc.tensor.matmul(
                        out=trp[:],
                        lhsT=out3T[:, c4 * 512 + j * 128:
                                   c4 * 512 + (j + 1) * 128],
                        rhs=diagS[:], start=True, stop=True)
                    if j % 2 == 0:
                        nc.scalar.copy(out=ev4[:, j * D:(j + 1) * D],
                                       in_=trp[:])
                    else:
                        nc.vector.tensor_copy(ev4[:, j * D:(j + 1) * D],
                                              trp[:])
                nc.sync.dma_start(
                    out=out_d[csl4, :].rearrange("(j p) d -> p j d", p=128),
                    in_=ev4[:].rearrange("p (j d) -> p j d", d=D))

    nc.compile()
    return nc


# ======================= runner =======================
_CACHE = {}


def _make_in_maps(per_core, shared):
    ebf = ml_dtypes.bfloat16
    in_maps = []
    for pc in per_core:
        in_maps.append(dict(
            srcgidT=pc['srcgidT'],
            OH=pc['OH'].astype(ebf),
            OHTW=pc['OHT'].astype(ebf),
            oh17=pc['oh17'].astype(ebf), n102=pc['n102'].astype(ebf),
            SELH2=shared['SELH2'].astype(ebf),
            T6A=shared['T6A'].astype(ebf), T6B=shared['T6B'].astype(ebf),
            outrowT=pc['outrowT'], npadvec=pc['npadvec'],
            embT=shared['embT'].astype(ebf),
            Wcat1=shared['Wcat1'].astype(ebf), Wcat2=shared['Wcat2'].astype(ebf),
            Wcat3=shared['Wcat3'].astype(ebf),
            W3stackA=shared['W3stackA'].astype(ebf),
            W3stackB=shared['W3stackB'].astype(ebf),
            r1=shared['r1'].astype(ebf), r2=shared['r2'].astype(ebf),
            r3=shared['r3'].astype(ebf),
            b1=shared['b1'], b2=shared['b2'], b3=shared['b3'],
            REP2=shared['REP2'], REP3A=shared['REP3A'], REP3B=shared['REP3B'],
            zrow=shared['zrow'].astype(ebf),
        ))
    return in_maps


def kernel(x, edge_index, edge_attr=None, **w):
    """Full inputs in, full [50000, 64] float32 softmax output out."""
    from concourse.bass_utils import run_bass_kernel_spmd
    args = dict(x=x, edge_index=edge_index)
    for k in ('emb', 'w1', 'as1', 'ad1', 'b1', 'r1', 'w2', 'as2', 'ad2', 'b2',
              'r2', 'w3', 'as3', 'ad3', 'b3', 'r3'):
        args[k] = np.asarray(w[k])
    per_core, shared, meta = host_prep(**args)
    key = (meta['TMAX'], meta['SLOTMAX'], meta['NMAXOUT'],
           shared['emb'].shape, shared['r2'].shape)
    if key not in _CACHE:
        _CACHE[key] = build_program(
            meta['TMAX'], meta['SLOTMAX'], meta['NMAXOUT'],
            shared['emb'].shape[1], shared['r2'].shape[0],
            shared['emb'].shape[0])
    nc = _CACHE[key]
    in_maps = _make_in_maps(per_core, shared)
    res = run_bass_kernel_spmd(nc, in_maps, list(range(NCORES)))
    D = shared['emb'].shape[1]
    N = meta['bnds'][-1]
    out = np.zeros((N, D), np.float32)
    for k in range(NCORES):
        nb = meta['bnds'][k]
        sn = meta['slot_node'][k]
        real = sn >= 0
        out[nb + sn[real]] = res.results[k]['out'][real]
    return out



# revision 78
# speedup vs baseline: 1.5285x; 1.0046x over previous
"""Trainium2 Bass kernel for nn_DiffusionOrderingNetwork (3-layer GAT, N=50000,
E=800000, softmax over nodes), SPMD across 8 NeuronCores.

Self-contained: host-side index/layout prep + Bass/Tile program + runner.
"""
import sys
sys.path.insert(0, '/opt/trn_rl_repo')
import numpy as np
import ml_dtypes
from contextlib import ExitStack

# ======================= host prep =======================
import numpy as _np

N = 50000
E = 800000
H = 6
C1 = 6
HID = 36
D = 64
NT = 17
NEG = 0.2
NCORES = 8
EPT = 128          # edges per tile
SPT = 8            # node slots per tile
KSUP = 32          # tiles per super-block (layers 1/2) -> 256 psum cols
KSUP3 = 16         # tiles per super-block (layer 3)   -> 128 psum cols


def _fold_ws(W, a):
    # ws[d, h] = sum_c W[d, h*C+c] * a[h, c]
    h, c = a.shape
    return np.einsum('dhc,hc->dh', W.reshape(W.shape[0], h, c), a).astype(np.float32)


def host_prep(x, edge_index, emb, w1, as1, ad1, b1, r1,
              w2, as2, ad2, b2, r2, w3, as3, ad3, b3, r3):
    x = np.asarray(x).astype(np.int64)
    ei = np.asarray(edge_index).astype(np.int64)
    N = x.shape[0]
    NT = emb.shape[0]
    D = emb.shape[1]
    for b in (b1, b2, b3):
        assert np.abs(np.asarray(b)).max() == 0.0, "nonzero bias breaks pad-column math"

    # --- edges sorted by dst; self-loops handled analytically on-device ---
    src = ei[0].copy()
    dst = ei[1].copy()
    order = np.argsort(dst, kind='stable')
    srcs = src[order]
    dsts = dst[order]
    ET = srcs.shape[0]
    deg = np.bincount(dst, minlength=N).astype(np.int64)
    assert deg.max() <= EPT, deg.max()
    node_ptr = np.concatenate([[0], np.cumsum(deg)])  # edge range per node

    # --- shard nodes into NCORES contiguous ranges with ~equal edges ---
    cum = np.cumsum(deg)
    bnds = [0]
    for k in range(1, NCORES):
        bnds.append(int(np.searchsorted(cum, ET * k / NCORES)))
    bnds.append(N)

    # --- per-core straddle tiling: a node's edges may split across two
    # adjacent tiles (never across a 16-tile super boundary); each tile has
    # at most SPT node starts; tiles fill to exactly EPT edges ---
    SB = 16
    core_tiles = []  # per core: list of tiles; tile = list of (node, e0, e1)
    for k in range(NCORES):
        nb, ne = bnds[k], bnds[k + 1]
        tiles = [[]]
        fill, starts = 0, 0
        for n in range(nb, ne):
            d = int(deg[n])
            e0 = int(node_ptr[n])
            if starts == SPT:
                tiles.append([])
                fill, starts = 0, 0
            starts += 1
            if d == 0:
                tiles[-1].append((n, e0, e0))
                continue
            rem = d
            while rem > 0:
                space = EPT - fill
                if space == 0:
                    tiles.append([])
                    fill, starts = 0, 1
                    space = EPT
                take = min(rem, space)
                if take < rem and (len(tiles) % SB) == 0:
                    # straddle would cross a super boundary: start fresh tile
                    tiles.append([])
                    fill, starts = 0, 1
                    take = min(rem, EPT)
                tiles[-1].append((n, e0, e0 + take))
                fill += take
                e0 += take
                rem -= take
        core_tiles.append(tiles)

    lcm = np.lcm(KSUP, np.lcm(KSUP3, 512 // SPT))  # tiles multiple for chunking
    TMAX = max(len(t) for t in core_tiles)
    TMAX = int(-(-TMAX // lcm) * lcm)
    SLOTMAX = TMAX * SPT
    NMAXOUT = max(bnds[k + 1] - bnds[k] for k in range(NCORES))
    NMAXOUT = int(-(-NMAXOUT // 128) * 128)

    # --- global slot-gid map (0 = poison row, then core-major slots);
    # a straddled node's slot lives in the tile of its FIRST edge segment ---
    nodeslot = np.zeros(N, dtype=np.int64)
    node_core = np.zeros(N, dtype=np.int64)
    core_slotof = [dict() for _ in range(NCORES)]  # node -> (tile, slot)
    for k in range(NCORES):
        slotof = core_slotof[k]
        for t, segs in enumerate(core_tiles[k]):
            nstart = 0
            for (n, e0, e1) in segs:
                if n not in slotof:
                    slotof[n] = (t, nstart)
                    nstart += 1
        for n, (t, s) in slotof.items():
            nodeslot[n] = 1 + k * SLOTMAX + t * SPT + s
            node_core[n] = k

    # --- per-core device input arrays ---
    # OHW/OHTW carry a 2-tile slot window per tile: cols/rows [0:SPT) are the
    # PREVIOUS tile's slots (for straddled nodes), [SPT:2*SPT) this tile's.
    W2 = 2 * SPT
    per_core = []
    for k in range(NCORES):
        tiles = core_tiles[k]
        nb, ne = bnds[k], bnds[k + 1]
        nreal = ne - nb
        slotof = core_slotof[k]

        srcgidT = np.zeros((EPT, TMAX), dtype=np.int32)
        OHW = np.zeros((EPT, TMAX * W2), dtype=np.float32)
        OHTW = np.zeros((W2, TMAX * EPT), dtype=np.float32)
        oh17 = np.zeros((NT, SLOTMAX), dtype=np.float32)
        n102 = np.zeros((H * NT, SLOTMAX), dtype=np.float32)
        outrowT = np.zeros((128, SLOTMAX // 128), dtype=np.int32) + 10**7
        slot_node = np.full(SLOTMAX, -1, dtype=np.int64)

        for t, segs in enumerate(tiles):
            row = 0
            for (n, e0, e1) in segs:
                ts, ss = slotof[n]
                assert ts in (t - 1, t)
                wcol = ss + (SPT if ts == t else 0)
                nseg = e1 - e0
                rows = row + np.arange(nseg)
                srcgidT[rows, t] = nodeslot[srcs[e0:e1]]
                OHW[rows, t * W2 + wcol] = 1.0
                OHTW[wcol, t * EPT + rows] = 1.0
                np.add.at(n102, (x[srcs[e0:e1]],
                                 np.full(nseg, ts * SPT + ss)), 1.0)
                row += nseg
                if ts == t:
                    sl = t * SPT + ss
                    oh17[x[n], sl] = 1.0
                    outrowT[sl % 128, sl // 128] = n - nb
                    slot_node[sl] = n - nb
            assert row <= EPT
        # nodes whose slot-tile had no edge segment in that tile (zero-deg or
        # straddle-only) are covered above since slots come from first segs.
        # self-loop counts for the layer-0 histogram:
        own = np.arange(nb, ne)
        sl_own = (nodeslot[own] - 1 - k * SLOTMAX).astype(np.int64)
        np.add.at(n102, (x[own], sl_own), 1.0)
        n102 = np.tile(n102[:NT], (H, 1))

        npadvec = np.full((D, 1), SLOTMAX - nreal, dtype=np.float32)
        per_core.append(dict(
            srcgidT=srcgidT, OH=OHW, OHT=OHTW, oh17=oh17, n102=n102,
            outrowT=outrowT, npadvec=npadvec, nreal=nreal, nb=nb, ne=ne,
            slot_node=slot_node,
        ))

    # --- folded weights (shared across cores) ---
    f32 = np.float32
    Wcat1 = np.concatenate([_fold_ws(w1, as1), w1.astype(f32), _fold_ws(w1, ad1)], axis=1)
    Wcat2 = np.concatenate([_fold_ws(w2, as2), w2.astype(f32), _fold_ws(w2, ad2)], axis=1)
    # layer 3: records carry xin itself (identity block); xs3 scores fold w3/as3
    Wcat3 = np.concatenate([_fold_ws(w3, as3), np.eye(HID, dtype=f32), _fold_ws(w3, ad3)], axis=1)
    # W3stack[h*HID+c, o] = w3[c, h*D+o] / H   (mean over heads folded in)
    W3stack = (w3.reshape(HID, H, D).transpose(1, 0, 2).reshape(H * HID, D) / H).astype(f32)
    REP2 = np.zeros((H, HID), dtype=f32)
    REP2[np.arange(HID) // C1, np.arange(HID)] = 1.0
    d3 = np.arange(H * HID)
    REP3A = np.zeros((H, 128), dtype=f32)
    REP3A[d3[:128] // HID, np.arange(128)] = 1.0
    REP3B = np.zeros((H, H * HID - 128), dtype=f32)
    REP3B[d3[128:] // HID, np.arange(H * HID - 128)] = 1.0
    zrow = np.zeros((SPT, 48), dtype=f32)
    # head selector for the (h, t)-major layer-0 histogram rows
    SELH2 = np.zeros((H * NT, H), dtype=f32)
    SELH2[np.arange(H * NT), np.arange(H * NT) // NT] = 1.0
    # head-replication of hidden features for the layer-3 self-term
    T6A = np.zeros((HID, 128), dtype=f32)
    T6A[np.arange(128) % HID, np.arange(128)] = 1.0
    T6B = np.zeros((HID, H * HID - 128), dtype=f32)
    T6B[np.arange(128, H * HID) % HID, np.arange(H * HID - 128)] = 1.0

    shared = dict(
        emb=emb.astype(f32), embT=emb.astype(f32).T.copy(),
        Wcat1=Wcat1, Wcat2=Wcat2, Wcat3=Wcat3,
        W3stackA=W3stack[:128].copy(), W3stackB=W3stack[128:].copy(),
        r1=r1.astype(f32), r2=r2.astype(f32), r3=r3.astype(f32),
        b1=b1.astype(f32).reshape(-1, 1), b2=b2.astype(f32).reshape(-1, 1),
        b3=b3.astype(f32).reshape(-1, 1),
        REP2=REP2, REP3A=REP3A, REP3B=REP3B, zrow=zrow, SELH2=SELH2,
        T6A=T6A, T6B=T6B,
    )
    meta = dict(TMAX=TMAX, SLOTMAX=SLOTMAX, NMAXOUT=NMAXOUT,
                bnds=bnds, nreal=[pc['nreal'] for pc in per_core],
                slot_node=[pc['slot_node'] for pc in per_core])
    return per_core, shared, meta


def numpy_reference(x, edge_index, emb, w1, as1, ad1, b1, r1,
                    w2, as2, ad2, b2, r2, w3, as3, ad3, b3, r3):
    """Plain numpy port of reference.py for quick host validation."""
    def gat(xf, src, dst, W, a_s, a_d, b, r, concat):
        n = xf.shape[0]
        h, c = a_s.shape
        xs = (xf @ W).reshape(n, h, c)
        a_src = (xs * a_s).sum(-1)
        a_dst = (xs * a_d).sum(-1)
        e = a_src[src] + a_dst[dst]
        e = np.where(e > 0, e, NEG * e)
        m = np.full((n, h), -np.inf)
        np.maximum.at(m, dst, e)
        m = np.where(np.isfinite(m), m, 0.0)
        ex = np.exp(e - m[dst])
        s = np.zeros((n, h))
        np.add.at(s, dst, ex)
        alpha = ex / (s[dst] + 1e-16)
        out = np.zeros((n, h, c))
        np.add.at(out, dst, xs[src] * alpha[:, :, None])
        out = out.reshape(n, h * c) if concat else out.mean(1)
        return out + xf @ r + b

    hf = emb[np.asarray(x).astype(np.int64)]
    loops = np.arange(x.shape[0])
    src = np.concatenate([edge_index[0], loops])
    dst = np.concatenate([edge_index[1], loops])
    hf = np.maximum(gat(hf, src, dst, w1, as1, ad1, b1, r1, True), 0)
    hf = np.maximum(gat(hf, src, dst, w2, as2, ad2, b2, r2, True), 0)
    hf = gat(hf, src, dst, w3, as3, ad3, b3, r3, False)
    hf = hf - hf.max(0, keepdims=True)
    e = np.exp(hf)
    return (e / e.sum(0, keepdims=True)).astype(np.float32)


# ======================= device program =======================

import concourse.bass as bass
import concourse.tile as tile
from concourse import bacc, mybir
from concourse.masks import make_identity
from concourse.tile import add_dep_helper

F32 = mybir.dt.float32
I32 = mybir.dt.int32
BF16 = mybir.dt.bfloat16

H = 6
EPT = 128
SPT = 8
KSUP = 32      # tiles per super for layers 1/2 (256 psum cols)
KSUP3 = 16     # tiles per super for layer 3  (128 psum cols)


def build_program(TMAX, SLOTMAX, NMAXOUT, D, HID, NT, n_cores=8, edge_dt=BF16,
                  debug_dump=False):
    RW = 48                      # record row: asrc(6) | xs(HID=36) | adst(6)
    NCH128 = SLOTMAX // 128
    NCH512 = SLOTMAX // 512
    TROWS = 1 + n_cores * SLOTMAX
    V216 = H * HID               # 216
    VA = 128                     # layer-3 agg split A (dims 0:128)
    VB = V216 - 128              # 88
    cores = list(range(n_cores))

    nc = bacc.Bacc("TRN2", target_bir_lowering=False, debug=False,
                   num_devices=n_cores)

    def din(name, shape, dt=F32):
        return nc.dram_tensor(name, list(shape), dt, kind="ExternalInput")

    W2 = 2 * SPT
    srcg_d = din("srcgidT", [EPT, TMAX], I32)
    oh_d = din("OH", [EPT, TMAX * W2], edge_dt)
    ohtw_d = din("OHTW", [W2, TMAX * EPT], edge_dt)
    oh17_d = din("oh17", [NT, SLOTMAX], edge_dt)
    n102_d = din("n102", [H * NT, SLOTMAX], edge_dt)
    selh2_d = din("SELH2", [H * NT, H], edge_dt)
    t6a_d = din("T6A", [HID, VA], edge_dt)
    t6b_d = din("T6B", [HID, VB], edge_dt)
    outr_d = din("outrowT", [128, NCH128], I32)
    npad_d = din("npadvec", [D, 1])
    embt_d = din("embT", [D, NT], edge_dt)
    wcat_d = [din("Wcat1", [D, RW], edge_dt), din("Wcat2", [HID, RW], edge_dt),
              din("Wcat3", [HID, RW], edge_dt)]
    w3a_d = din("W3stackA", [VA, D], edge_dt)
    w3b_d = din("W3stackB", [VB, D], edge_dt)
    r_d = [din("r1", [D, HID], edge_dt), din("r2", [HID, HID], edge_dt),
           din("r3", [HID, D], edge_dt)]
    b_d = [din("b1", [HID, 1]), din("b2", [HID, 1]), din("b3", [D, 1])]
    rep2_d = din("REP2", [H, HID])
    rep3a_d = din("REP3A", [H, VA])
    rep3b_d = din("REP3B", [H, VB])
    zrow_d = din("zrow", [SPT, RW], edge_dt)
    out_d = nc.dram_tensor("out", [SLOTMAX, D], F32, kind="ExternalOutput")
    if debug_dump:
        dbg1_d = nc.dram_tensor("dbg1", [HID, SLOTMAX], edge_dt, kind="ExternalOutput")
        dbg2_d = nc.dram_tensor("dbg2", [HID, SLOTMAX], edge_dt, kind="ExternalOutput")
        dbg3_d = nc.dram_tensor("dbg3", [D, SLOTMAX], edge_dt, kind="ExternalOutput")

    ag_in = nc.dram_tensor("ag_in", [SLOTMAX, RW], edge_dt)
    table = nc.dram_tensor("table", [TROWS, RW], edge_dt)
    adstL = nc.dram_tensor("adstL", [SPT + SLOTMAX, H], edge_dt)
    cca_i = nc.dram_tensor("cca_i", [D, 1], F32)
    cca_o = nc.dram_tensor("cca_o", [D, 1], F32)
    ccs_i = nc.dram_tensor("ccs_i", [D, 1], F32)
    ccs_o = nc.dram_tensor("ccs_o", [D, 1], F32)

    with ExitStack() as ctx:
        tc = ctx.enter_context(tile.TileContext(nc))
        res = ctx.enter_context(tc.tile_pool(name="res", bufs=1))
        cst = ctx.enter_context(tc.tile_pool(name="cst", bufs=1))
        aux = ctx.enter_context(tc.tile_pool(name="aux", bufs=2, space="PSUM"))
        p1p = ctx.enter_context(tc.tile_pool(name="p1p", bufs=3))

        def load(pool, src, shape, dt=F32, tag=None):
            t = pool.tile(list(shape), dt, tag=tag)
            nc.sync.dma_start(out=t[:], in_=src[:])
            return t

        srcg = res.tile([EPT, TMAX], I32, tag="srcg")
        oht_sb = res.tile([EPT, TMAX * W2], edge_dt, tag="oht")
        outr = load(cst, outr_d, [128, NCH128], I32, tag="outr")
        t6a_sb = load(cst, t6a_d, [HID, VA], edge_dt, tag="t6a")
        t6b_sb = load(cst, t6b_d, [HID, VB], edge_dt, tag="t6b")
        npad_sb = load(cst, npad_d, [D, 1], tag="npad")
        embt_sb = load(cst, embt_d, [D, NT], edge_dt, tag="embt")
        selh2_sb = load(cst, selh2_d, [H * NT, H], edge_dt, tag="selh2")
        wcat_sb = [load(cst, wcat_d[0], [D, RW], edge_dt, tag="wc1"),
                   load(cst, wcat_d[1], [HID, RW], edge_dt, tag="wc2"),
                   load(cst, wcat_d[2], [HID, RW], edge_dt, tag="wc3")]
        w3a_sb = load(cst, w3a_d, [VA, D], edge_dt, tag="w3a")
        w3b_sb = load(cst, w3b_d, [VB, D], edge_dt, tag="w3b")
        r_sb = [load(cst, r_d[0], [D, HID], edge_dt, tag="r1"),
                load(cst, r_d[1], [HID, HID], edge_dt, tag="r2"),
                load(cst, r_d[2], [HID, D], edge_dt, tag="r3")]
        b_sb = [load(cst, b_d[0], [HID, 1], tag="b1"),
                load(cst, b_d[1], [HID, 1], tag="b2"),
                load(cst, b_d[2], [D, 1], tag="b3")]
        rep2_sb = load(cst, rep2_d, [H, HID], tag="rep2")
        rep3a_sb = load(cst, rep3a_d, [H, VA], tag="rep3a")
        rep3b_sb = load(cst, rep3b_d, [H, VB], tag="rep3b")
        idn = cst.tile([64, 64], edge_dt, tag="idn")
        make_identity(nc, idn[:])
        nc.sync.dma_start(out=table[0:1, :], in_=zrow_d[0:1, :])
        adz = nc.sync.dma_start(out=adstL[0:SPT, :], in_=zrow_d[:, 0:H])

        # ---- t17 = per-type layer-1 records [NT, RW]; er1 = emb @ r1 ----
        V102 = H * NT
        t17_sb = cst.tile([NT, RW], edge_dt, tag="t17")
        t17f_sb = cst.tile([NT, RW], F32, tag="t17f")
        er1_sb = cst.tile([NT, HID], edge_dt, tag="er1")
        with tc.tile_pool(name="p17", bufs=1, space="PSUM") as p17:
            ps = p17.tile([NT, RW], F32, space="PSUM", tag="ps")
            nc.tensor.matmul(out=ps[:], lhsT=embt_sb[:], rhs=wcat_sb[0][:],
                             start=True, stop=True)
            nc.vector.tensor_copy(t17_sb[:], ps[:])
            nc.vector.tensor_copy(t17f_sb[:], ps[:])
            pse = p17.tile([NT, HID], F32, space="PSUM", tag="pse")
            nc.tensor.matmul(out=pse[:], lhsT=embt_sb[:], rhs=r_sb[0][:],
                             start=True, stop=True)
            nc.vector.tensor_copy(er1_sb[:], pse[:])

        # ---- layer-0 histogram operands derived from t17 ----
        # rows are (h, t)-major: row h*NT+t
        L_sb = cst.tile([NT, V102], edge_dt, tag="Lsb")       # ad expander
        at102 = cst.tile([V102, 1], F32, tag="at102")         # a_src per (h,t)
        w17t = cst.tile([V102, HID], edge_dt, tag="w17t")     # xs selector
        nc.vector.memset(w17t[:], 0.0)
        for h in range(H):
            nc.vector.tensor_copy(
                L_sb[:, h * NT:(h + 1) * NT],
                t17_sb[:, RW - H + h:RW - H + h + 1].to_broadcast([NT, NT]))
            # partition-shifted moves must go through DMA, not DVE
            nc.sync.dma_start(out=at102[h * NT:(h + 1) * NT, :],
                              in_=t17f_sb[:, h:h + 1])
            c0 = H + h * (HID // H)
            nc.sync.dma_start(
                out=w17t[h * NT:(h + 1) * NT,
                         h * (HID // H):(h + 1) * (HID // H)],
                in_=t17_sb[:, c0:c0 + HID // H])

        lsum = cst.tile([D, 1], F32, tag="lsum")
        lsum2 = cst.tile([D, 1], F32, tag="lsum2")
        # summed (asrc + adst) weight columns for the self-loop terms
        wsum1 = cst.tile([HID, H], edge_dt, tag="wsum1")
        wsum2 = cst.tile([HID, H], edge_dt, tag="wsum2")
        wsum_sb = [wsum1, wsum2]
        for i in (0, 1):
            nc.vector.tensor_tensor(out=wsum_sb[i][:],
                                    in0=wcat_sb[i + 1][:, 0:H],
                                    in1=wcat_sb[i + 1][:, RW - H:RW],
                                    op=mybir.AluOpType.add)

        hT1 = res.tile([HID, SLOTMAX], edge_dt, tag="h36a")
        hT2 = res.tile([HID, SLOTMAX], edge_dt, tag="h36b")
        out3T = res.tile([D, SLOTMAX], edge_dt, tag="h64")
        agg3A = res.tile([VA, SLOTMAX], edge_dt, tag="agg3A")
        agg3B = res.tile([VB, SLOTMAX], edge_dt, tag="agg3B")
        h6a_sb = res.tile([VA, SLOTMAX], edge_dt, tag="h6a")
        h6b_sb = res.tile([VB, SLOTMAX], edge_dt, tag="h6b")

        # pipelined record-phase: emit one 128-col record chunk for layer l
        hmap = {1: hT1, 2: hT2}
        adw_map = {1: [], 2: []}
        agst = {'cc': None}

        def emit_p1(l, c4):
            # one 512-slot group: 4 record matmuls, one sb tile, 2 DMAs
            sb4 = p1p.tile([128, 4 * RW], edge_dt, tag="sb")
            for j in range(4):
                c = 4 * c4 + j
                ps1 = aux.tile([128, 512], F32, space="PSUM", tag="aux")
                nc.tensor.matmul(out=ps1[:, 0:RW],
                                 lhsT=hmap[l][:, c * 128:(c + 1) * 128],
                                 rhs=wcat_sb[l][:], start=True, stop=True)
                nc.scalar.copy(out=sb4[:, j * RW:(j + 1) * RW],
                               in_=ps1[:, 0:RW])
            sb3 = sb4[:].rearrange("p (j d) -> p j d", d=RW)
            wdma = nc.sync.dma_start(
                out=ag_in[c4 * 512:(c4 + 1) * 512,
                          :].rearrange("(j p) d -> p j d", p=128),
                in_=sb3)
            if agst['cc'] is not None:
                for _cc in agst['cc']:
                    add_dep_helper(wdma.ins, _cc.ins,
                                   reason="ag_in WAR vs previous AllGather")
            adw = nc.sync.dma_start(
                out=adstL[SPT + c4 * 512:SPT + (c4 + 1) * 512,
                          :].rearrange("(j p) d -> p j d", p=128),
                in_=sb3[:, :, RW - H:RW])
            adw_map[l].append(adw)

        # ---- layer 0: per-slot type-histogram GAT (no per-edge work) ----
        with tc.tile_pool(name="l0", bufs=3) as p0, \
             tc.tile_pool(name="l0in", bufs=1) as pin, \
             tc.tile_pool(name="l0a", bufs=2, space="PSUM") as pA, \
             tc.tile_pool(name="l0b", bufs=1, space="PSUM") as pB, \
             tc.tile_pool(name="l0c", bufs=1, space="PSUM") as pC:
            oh17_sb = pin.tile([NT, SLOTMAX], edge_dt, tag="oh17s")
            n102_sb = pin.tile([V102, SLOTMAX], edge_dt, tag="n102s")
            hsm = SLOTMAX // 2
            nc.sync.dma_start(out=oh17_sb[:, 0:hsm], in_=oh17_d[:, 0:hsm])
            nc.sync.dma_start(out=n102_sb[:, 0:hsm], in_=n102_d[:, 0:hsm])
            nc.sync.dma_start(out=oh17_sb[:, hsm:], in_=oh17_d[:, hsm:])
            nc.sync.dma_start(out=n102_sb[:, hsm:], in_=n102_d[:, hsm:])
            for c in range(NCH512):
                csl = slice(c * 512, (c + 1) * 512)
                ohc = oh17_sb[:, csl]
                n102c = n102_sb[:, csl]
                ps102 = pA.tile([V102, 512], F32, space="PSUM", tag="ps102")
                nc.tensor.matmul(out=ps102[:], lhsT=L_sb[:], rhs=ohc,
                                 start=True, stop=True)
                esc = p0.tile([V102, 512], F32, tag="esc0")
                nc.vector.tensor_scalar_add(out=esc[:], in0=ps102[:],
                                            scalar1=at102[:])
                nc.vector.scalar_tensor_tensor(
                    out=esc[:], in0=esc[:], scalar=0.2, in1=esc[:],
                    op0=mybir.AluOpType.mult, op1=mybir.AluOpType.max)
                nc.scalar.activation(out=esc[:], in_=esc[:],
                                     func=mybir.ActivationFunctionType.Exp)
                nE = p0.tile([V102, 512], edge_dt, tag="nE")
                nc.gpsimd.tensor_tensor(out=nE[:], in0=esc[:], in1=n102c,
                                        op=mybir.AluOpType.mult)
                psD = pC.tile([H, 512], F32, space="PSUM", tag="psD")
                nc.tensor.matmul(out=psD[:], lhsT=selh2_sb[:], rhs=nE[:],
                                 start=True, stop=True)
                psN = pB.tile([HID, 512], F32, space="PSUM", tag="psN")
                nc.tensor.matmul(out=psN[:], lhsT=w17t[:], rhs=nE[:],
                                 start=True, stop=True)
                psR = pB.tile([HID, 512], F32, space="PSUM", tag="psR")
                nc.tensor.matmul(out=psR[:], lhsT=er1_sb[:], rhs=ohc,
                                 start=True, stop=True)
                rs = p0.tile([H, 512], F32, tag="rs0")
                nc.vector.tensor_scalar_add(out=rs[:], in0=psD[:],
                                            scalar1=1e-16)
                nc.vector.reciprocal(out=rs[:], in_=rs[:])
                ps2 = pC.tile([HID, 512], F32, space="PSUM", tag="ps20")
                nc.tensor.matmul(out=ps2[:], lhsT=rep2_sb[:], rhs=rs[:],
                                 start=True, stop=True)
                rr = p0.tile([HID, 512], F32, tag="rr0")
                nc.scalar.copy(out=rr[:], in_=ps2[:])
                nc.vector.tensor_tensor(out=hT1[:, csl], in0=psN[:], in1=rr[:],
                                        op=mybir.AluOpType.mult)
                nc.vector.tensor_tensor(out=hT1[:, csl], in0=hT1[:, csl],
                                        in1=psR[:], op=mybir.AluOpType.add)
                nc.scalar.activation(out=hT1[:, csl], in_=hT1[:, csl],
                                     func=mybir.ActivationFunctionType.Relu,
                                     bias=b_sb[0][:])
                emit_p1(1, c)
        nc.sync.dma_start(out=srcg[:], in_=srcg_d[:])
        nc.sync.dma_start(out=oht_sb[:, 0:TMAX * SPT], in_=oh_d[:, 0:TMAX * SPT])
        nc.sync.dma_start(out=oht_sb[:, TMAX * SPT:], in_=oh_d[:, TMAX * SPT:])
        if debug_dump:
            nc.sync.dma_start(out=dbg1_d[:], in_=hT1[:])

        hins = [None, hT1, hT2]
        houts = [None, hT2, None]
        prev_cc = None
        prev_readers = []

        for l in (1, 2):
            hin = hins[l]
            adst_writes = adw_map[l]

            # ---- P2: all-gather the record table (written by pipelined P1) ----
            if n_cores == 1:
                # model the collective as 8 parallel chunk copies
                ccs_l = []
                nch8 = SLOTMAX // 8
                for i8 in range(8):
                    cci = nc.sync.dma_start(
                        out=table[1 + i8 * nch8:1 + (i8 + 1) * nch8, :],
                        in_=ag_in[i8 * nch8:(i8 + 1) * nch8, :])
                    ccs_l.append(cci)
            else:
                ccs_l = [nc.gpsimd.collective_compute(
                    "AllGather", mybir.AluOpType.bypass,
                    replica_groups=[cores],
                    ins=[ag_in[:]], outs=[table[1:, :]],
                )]
            for cc in ccs_l:
                for rd in prev_readers:
                    add_dep_helper(cc.ins, rd.ins,
                                   reason="table WAR vs previous layer gathers")
            prev_cc = ccs_l
            agst['cc'] = ccs_l
            prev_readers = []

            # ---- P3: edge phase (scatter windows are 2 tiles wide: a node
            # may straddle into the next tile; psum accumulates) ----
            ks = KSUP if l < 2 else KSUP3
            nsup = TMAX // ks
            lw = RW - H if l < 2 else H + V216   # scatter lhsT width: 42 / 222
            cols = ks * SPT                      # real psum cols per super
            pcols = cols + SPT                   # + leading ghost window
            with tc.tile_pool(name=f"ed{l}", bufs=3) as wp, \
                 tc.tile_pool(name=f"edp{l}", bufs=1, space="PSUM") as pp, \
                 tc.tile_pool(name=f"eds{l}", bufs=1, space="PSUM") as pps, \
                 tc.tile_pool(name=f"tmp{l}", bufs=1, space="PSUM") as tpp, \
                 tc.tile_pool(name=f"rcp{l}", bufs=1, space="PSUM") as tpr, \
                 tc.tile_pool(name=f"adp{l}", bufs=1, space="PSUM") as adp:
                for g in range(nsup):
                    t0 = g * ks
                    csl = slice(g * cols, (g + 1) * cols)
                    Rg = wp.tile([EPT, ks * RW], edge_dt, tag="Rg")
                    for k in range(ks):
                        gi = nc.gpsimd.indirect_dma_start(
                            out=Rg[:, k * RW:(k + 1) * RW],
                            out_offset=None, in_=table[:],
                            in_offset=bass.IndirectOffsetOnAxis(
                                ap=srcg[:, t0 + k:t0 + k + 1], axis=0))
                        for _cc in prev_cc:
                            add_dep_helper(gi.ins, _cc.ins,
                                           reason="gather RAW AllGather")
                        prev_readers.append(gi)
                    # a_dst expansion operands: 16-row window = prev|own slots
                    ohts = wp.tile([W2, ks * EPT], edge_dt, tag="ohts")
                    nc.sync.dma_start(out=ohts[:],
                                      in_=ohtw_d[:, t0 * EPT:(t0 + ks) * EPT])
                    adsw = wp.tile([W2, ks * H], edge_dt, tag="adsw")
                    adr0 = nc.sync.dma_start(
                        out=adsw[0:SPT, :].rearrange("s (k e) -> s k e", e=H),
                        in_=adstL[t0 * SPT:(t0 + ks) * SPT, :].rearrange(
                            "(k s) e -> s k e", s=SPT))
                    adr1 = nc.sync.dma_start(
                        out=adsw[SPT:W2, :].rearrange("s (k e) -> s k e", e=H),
                        in_=adstL[(t0 + 1) * SPT:(t0 + ks + 1) * SPT,
                                  :].rearrange("(k s) e -> s k e", s=SPT))
                    for c in range(max(0, (t0 * SPT - SPT)) // 512,
                                   ((t0 + ks) * SPT + 511) // 512):
                        add_dep_helper(adr0.ins, adst_writes[c].ins,
                                       reason="ads RAW adstL chunk write")
                        add_dep_helper(adr1.ins, adst_writes[c].ins,
                                       reason="ads RAW adstL chunk write")
                    if g == 0:
                        add_dep_helper(adr0.ins, adz.ins,
                                       reason="ads RAW adstL zero rows")
                    psAD = adp.tile([EPT, ks * H], F32, space="PSUM", tag="psAD")
                    for k in range(ks):
                        nc.tensor.matmul(
                            out=psAD[:, k * H:(k + 1) * H],
                            lhsT=ohts[:, k * EPT:(k + 1) * EPT],
                            rhs=adsw[:, k * H:(k + 1) * H],
                            start=True, stop=True)
                    R3 = Rg[:].rearrange("p (k e) -> p k e", e=RW)
                    esc = wp.tile([EPT, ks * H], F32, tag="esc")
                    nc.vector.tensor_tensor(
                        out=esc[:], in0=R3[:, :, 0:H],
                        in1=psAD[:], op=mybir.AluOpType.add)
                    nc.vector.scalar_tensor_tensor(
                        out=esc[:], in0=esc[:], scalar=0.2, in1=esc[:],
                        op0=mybir.AluOpType.mult, op1=mybir.AluOpType.max)
                    RHS = wp.tile([EPT, ks * lw], edge_dt, tag="RHS")
                    S3 = RHS[:].rearrange("p (k e) -> p k e", e=lw)
                    nc.scalar.activation(
                        out=S3[:, :, 0:H],
                        in_=esc[:].rearrange("p (k h) -> p k h", h=H),
                        func=mybir.ActivationFunctionType.Exp)
                    ex_rep = S3[:, :, 0:H][:, :, :, None].to_broadcast(
                        [EPT, ks, H, lw // H - 1])
                    if l < 2:
                        xs_in = R3[:, :, H:RW - H].rearrange(
                            "p k (h c) -> p k h c", h=H)
                    else:
                        xs_in = R3[:, :, H:RW - H][:, :, None, :].to_broadcast(
                            [EPT, ks, H, HID])
                    nc.vector.tensor_tensor(
                        out=S3[:, :, H:lw].rearrange("p k (h c) -> p k h c", h=H),
                        in0=xs_in, in1=ex_rep, op=mybir.AluOpType.mult)
                    # self-loop term: esc_self = hin.T @ (W_asrc + W_adst)
                    psRec = tpr.tile([H, cols], F32, space="PSUM",
                                     tag="psRec")
                    nc.tensor.matmul(out=psRec[:], lhsT=wsum_sb[l - 1][:],
                                     rhs=hin[:, csl], start=True, stop=True)
                    e3 = wp.tile([H, cols], F32, tag="e3")
                    nc.scalar.copy(out=e3[:], in_=psRec[:])
                    nc.vector.scalar_tensor_tensor(
                        out=e3[:], in0=e3[:], scalar=0.2, in1=e3[:],
                        op0=mybir.AluOpType.mult, op1=mybir.AluOpType.max)
                    nc.scalar.activation(out=e3[:], in_=e3[:],
                                         func=mybir.ActivationFunctionType.Exp)
                    # scatter with 16-col windows, accumulating
                    psS = (pp if l < 2 else pps).tile([H, pcols], F32,
                                                      space="PSUM", tag="psS")
                    nc.vector.memset(psS[:], 0.0)
                    if l < 2:
                        psV = pp.tile([HID, pcols], F32, space="PSUM", tag="psV")
                        nc.vector.memset(psV[:], 0.0)
                    else:
                        psA = pp.tile([VA, pcols], F32, space="PSUM", tag="psA")
                        psB = pp.tile([VB, pcols], F32, space="PSUM", tag="psB")
                        nc.vector.memset(psA[:], 0.0)
                        nc.vector.memset(psB[:], 0.0)
                    for k in range(ks):
                        t = t0 + k
                        ohs = oht_sb[:, t * W2:(t + 1) * W2]
                        wsl = slice(k * SPT, k * SPT + W2)
                        lb = k * lw
                        nc.tensor.matmul(
                            out=psS[:, wsl],
                            lhsT=RHS[:, lb:lb + H], rhs=ohs,
                            start=False, stop=True)
                        if l < 2:
                            nc.tensor.matmul(
                                out=psV[:, wsl],
                                lhsT=RHS[:, lb + H:lb + lw], rhs=ohs,
                                start=False, stop=True)
                        else:
                            nc.tensor.matmul(
                                out=psA[:, wsl],
                                lhsT=RHS[:, lb + H:lb + H + VA], rhs=ohs,
                                start=False, stop=True)
                            nc.tensor.matmul(
                                out=psB[:, wsl],
                                lhsT=RHS[:, lb + H + VA:lb + lw], rhs=ohs,
                                start=False, stop=True)
                    rs = wp.tile([H, cols], F32, tag="rs")
                    nc.vector.scalar_tensor_tensor(
                        out=rs[:], in0=psS[:, SPT:], scalar=1e-16, in1=e3[:],
                        op0=mybir.AluOpType.add, op1=mybir.AluOpType.add)
                    nc.vector.reciprocal(out=rs[:], in_=rs[:])
                    z = wp.tile([H, cols], F32, tag="z")
                    nc.vector.tensor_tensor(out=z[:], in0=e3[:], in1=rs[:],
                                            op=mybir.AluOpType.mult)
                    if l < 2:
                        ps2 = tpp.tile([HID, cols], F32, space="PSUM",
                                       tag="ps2")
                        nc.tensor.matmul(out=ps2[:], lhsT=rep2_sb[:], rhs=rs[:],
                                         start=True, stop=True)
                        rr = wp.tile([HID, cols], F32, tag="rr")
                        nc.scalar.copy(out=rr[:], in_=ps2[:])
                        psZ = tpp.tile([HID, cols], F32, space="PSUM",
                                       tag="ps2")
                        nc.tensor.matmul(out=psZ[:], lhsT=rep2_sb[:], rhs=z[:],
                                         start=True, stop=True)
                        zz = wp.tile([HID, cols], F32, tag="zz")
                        nc.scalar.copy(out=zz[:], in_=psZ[:])
                        # self value xs_self = wcat[:, H:RW-H].T @ hin
                        psXS = tpp.tile([HID, cols], F32, space="PSUM",
                                        tag="ps2")
                        nc.tensor.matmul(out=psXS[:],
                                         lhsT=wcat_sb[l][:, H:RW - H],
                                         rhs=hin[:, csl], start=True, stop=True)
                        nc.vector.tensor_tensor(
                            out=zz[:], in0=zz[:], in1=psXS[:],
                            op=mybir.AluOpType.mult)
                        nc.vector.tensor_tensor(
                            out=rr[:], in0=psV[:, SPT:], in1=rr[:],
                            op=mybir.AluOpType.mult)
                        nc.vector.tensor_tensor(
                            out=houts[l][:, csl], in0=rr[:], in1=zz[:],
                            op=mybir.AluOpType.add)
                    else:
                        for rep_sb, psX, h6, agg, vx in (
                                (rep3a_sb, psA, h6a_sb, agg3A, VA),
                                (rep3b_sb, psB, h6b_sb, agg3B, VB)):
                            ps2X = tpp.tile([vx, cols], F32, space="PSUM",
                                            tag="ps2")
                            nc.tensor.matmul(out=ps2X[:], lhsT=rep_sb[:],
                                             rhs=rs[:], start=True, stop=True)
                            rrX = wp.tile([vx, cols], F32, tag="rrX")
                            nc.scalar.copy(out=rrX[:], in_=ps2X[:])
                            psZX = tpp.tile([vx, cols], F32, space="PSUM",
                                            tag="ps2")
                            nc.tensor.matmul(out=psZX[:], lhsT=rep_sb[:],
                                             rhs=z[:], start=True, stop=True)
                            zzX = wp.tile([vx, cols], F32, tag="zzX")
                            nc.scalar.copy(out=zzX[:], in_=psZX[:])
                            nc.vector.tensor_tensor(
                                out=zzX[:], in0=zzX[:], in1=h6[:, csl],
                                op=mybir.AluOpType.mult)
                            nc.vector.tensor_tensor(
                                out=rrX[:], in0=psX[:, SPT:], in1=rrX[:],
                                op=mybir.AluOpType.mult)
                            nc.vector.tensor_tensor(
                                out=agg[:, csl], in0=rrX[:], in1=zzX[:],
                                op=mybir.AluOpType.add)

                    # ---- pipelined finalize of completed 512-col chunks ----
                    if l == 1 and g % 2 == 1:
                        c5 = g // 2
                        csl5 = slice(c5 * 512, (c5 + 1) * 512)
                        psr = aux.tile([128, 512], F32, space="PSUM", tag="aux")
                        nc.tensor.matmul(out=psr[0:HID, :], lhsT=r_sb[1][:],
                                         rhs=hT1[:, csl5], start=True, stop=True)
                        nc.vector.tensor_tensor(out=hT2[:, csl5],
                                                in0=hT2[:, csl5],
                                                in1=psr[0:HID, :],
                                                op=mybir.AluOpType.add)
                        nc.scalar.activation(
                            out=hT2[:, csl5], in_=hT2[:, csl5],
                            func=mybir.ActivationFunctionType.Relu,
                            bias=b_sb[1][:])
                        emit_p1(2, c5)
                        psh = aux.tile([128, 512], F32, space="PSUM", tag="aux")
                        nc.tensor.matmul(out=psh[0:VA, :], lhsT=t6a_sb[:],
                                         rhs=hT2[:, csl5], start=True, stop=True)
                        nc.scalar.copy(out=h6a_sb[:, csl5], in_=psh[0:VA, :])
                        psh2 = aux.tile([128, 512], F32, space="PSUM", tag="aux")
                        nc.tensor.matmul(out=psh2[0:VB, :], lhsT=t6b_sb[:],
                                         rhs=hT2[:, csl5], start=True, stop=True)
                        nc.scalar.copy(out=h6b_sb[:, csl5], in_=psh2[0:VB, :])
                    if l == 2 and g % 4 == 3:
                        c5 = g // 4
                        csl5 = slice(c5 * 512, (c5 + 1) * 512)
                        ps3 = aux.tile([128, 512], F32, space="PSUM", tag="aux")
                        nc.tensor.matmul(out=ps3[0:D, :], lhsT=w3a_sb[:],
                                         rhs=agg3A[:, csl5],
                                         start=True, stop=False)
                        nc.tensor.matmul(out=ps3[0:D, :], lhsT=w3b_sb[:],
                                         rhs=agg3B[:, csl5],
                                         start=False, stop=False)
                        nc.tensor.matmul(out=ps3[0:D, :], lhsT=r_sb[2][:],
                                         rhs=hT2[:, csl5],
                                         start=False, stop=True)
                        nc.vector.tensor_scalar_add(out=out3T[:, csl5],
                                                    in0=ps3[0:D, :],
                                                    scalar1=b_sb[2][:])
                        # logits are O(1): exp + sum need no max-subtraction
                        nc.scalar.activation(
                            out=out3T[:, csl5], in_=out3T[:, csl5],
                            func=mybir.ActivationFunctionType.Exp)
                        lsc = wp.tile([D, 1], F32, tag="lsc")
                        nc.vector.tensor_reduce(out=lsc[:],
                                                in_=out3T[:, csl5],
                                                axis=mybir.AxisListType.X,
                                                op=mybir.AluOpType.add)
                        acc = lsum if c5 % 2 == 0 else lsum2
                        if c5 < 2:
                            nc.vector.tensor_copy(acc[:], lsc[:])
                        else:
                            nc.vector.tensor_tensor(out=acc[:], in0=acc[:],
                                                    in1=lsc[:],
                                                    op=mybir.AluOpType.add)

            if debug_dump and l == 1:
                nc.sync.dma_start(out=dbg2_d[:], in_=hT2[:])

        # ---- P5: softmax over nodes (global across cores) ----
        with tc.tile_pool(name="sm", bufs=2) as sp, \
             tc.tile_pool(name="smp", bufs=2, space="PSUM") as spp:
            nc.vector.tensor_tensor(out=lsum[:], in0=lsum[:], in1=lsum2[:],
                                    op=mybir.AluOpType.add)
            # pad slots each contribute exp(0) = 1; subtract their count
            nc.vector.tensor_sub(out=lsum[:], in0=lsum[:], in1=npad_sb[:])
            nc.sync.dma_start(out=ccs_i[:], in_=lsum[:])
            if n_cores == 1:
                cc2 = nc.sync.dma_start(out=ccs_o[:], in_=ccs_i[:])
            else:
                cc2 = nc.gpsimd.collective_compute(
                    "AllReduce", mybir.AluOpType.add, replica_groups=[cores],
                    ins=[ccs_i[:]], outs=[ccs_o[:]])
            gsum = sp.tile([D, 1], F32, tag="gsum")
            rb2 = nc.sync.dma_start(out=gsum[:], in_=ccs_o[:])
            add_dep_helper(rb2.ins, cc2.ins, reason="read AllReduce sum result")
            nc.vector.reciprocal(out=gsum[:], in_=gsum[:])
            # fold 1/gsum into a diagonal matmul that also transposes
            diagS = sp.tile([64, 64], edge_dt, tag="diagS")
            nc.vector.tensor_scalar_mul(out=diagS[:], in0=idn[:],
                                        scalar1=gsum[:])
            if debug_dump:
                nc.vector.tensor_scalar_mul(out=out3T[:], in0=out3T[:],
                                            scalar1=gsum[:])
                nc.sync.dma_start(out=dbg3_d[:], in_=out3T[:])
            for c4 in range(NCH128 // 4):
                csl4 = slice(c4 * 512, (c4 + 1) * 512)
                ev4 = sp.tile([128, 4 * D], F32, tag="ev4")
                for j in range(4):
                    trp = spp.tile([128, D], F32, space="PSUM", tag="trp")
                    nc.tensor.matmul(
                        out=trp[:],
                        lhsT=out3T[:, c4 * 512 + j * 128:
                                   c4 * 512 + (j + 1) * 128],
                        rhs=diagS[:], start=True, stop=True)
                    if j % 2 == 0:
                        nc.scalar.copy(out=ev4[:, j * D:(j + 1) * D],
                                       in_=trp[:])
                    else:
                        nc.vector.tensor_copy(ev4[:, j * D:(j + 1) * D],
                                              trp[:])
                nc.sync.dma_start(
                    out=out_d[csl4, :].rearrange("(j p) d -> p j d", p=128),
                    in_=ev4[:].rearrange("p (j d) -> p j d", d=D))

    nc.compile()
    return nc


# ======================= runner =======================
_CACHE = {}


def _make_in_maps(per_core, shared):
    ebf = ml_dtypes.bfloat16
    in_maps = []
    for pc in per_core:
        in_maps.append(dict(
            srcgidT=pc['srcgidT'],
            OH=pc['OH'].astype(ebf),
            OHTW=pc['OHT'].astype(ebf),
            oh17=pc['oh17'].astype(ebf), n102=pc['n102'].astype(ebf),
            SELH2=shared['SELH2'].astype(ebf),
            T6A=shared['T6A'].astype(ebf), T6B=shared['T6B'].astype(ebf),
            outrowT=pc['outrowT'], npadvec=pc['npadvec'],
            embT=shared['embT'].astype(ebf),
            Wcat1=shared['Wcat1'].astype(ebf), Wcat2=shared['Wcat2'].astype(ebf),
            Wcat3=shared['Wcat3'].astype(ebf),
            W3stackA=shared['W3stackA'].astype(ebf),
            W3stackB=shared['W3stackB'].astype(ebf),
            r1=shared['r1'].astype(ebf), r2=shared['r2'].astype(ebf),
            r3=shared['r3'].astype(ebf),
            b1=shared['b1'], b2=shared['b2'], b3=shared['b3'],
            REP2=shared['REP2'], REP3A=shared['REP3A'], REP3B=shared['REP3B'],
            zrow=shared['zrow'].astype(ebf),
        ))
    return in_maps


def kernel(x, edge_index, edge_attr=None, **w):
    """Full inputs in, full [50000, 64] float32 softmax output out."""
    from concourse.bass_utils import run_bass_kernel_spmd
    args = dict(x=x, edge_index=edge_index)
    for k in ('emb', 'w1', 'as1', 'ad1', 'b1', 'r1', 'w2', 'as2', 'ad2', 'b2',
              'r2', 'w3', 'as3', 'ad3', 'b3', 'r3'):
        args[k] = np.asarray(w[k])
    per_core, shared, meta = host_prep(**args)
    key = (meta['TMAX'], meta['SLOTMAX'], meta['NMAXOUT'],
           shared['emb'].shape, shared['r2'].shape)
    if key not in _CACHE:
        _CACHE[key] = build_program(
            meta['TMAX'], meta['SLOTMAX'], meta['NMAXOUT'],
            shared['emb'].shape[1], shared['r2'].shape[0],
            shared['emb'].shape[0])
    nc = _CACHE[key]
    in_maps = _make_in_maps(per_core, shared)
    res = run_bass_kernel_spmd(nc, in_maps, list(range(NCORES)))
    D = shared['emb'].shape[1]
    N = meta['bnds'][-1]
    out = np.zeros((N, D), np.float32)
    for k in range(NCORES):
        nb = meta['bnds'][k]
        sn = meta['slot_node'][k]
        real = sn >= 0
        out[nb + sn[real]] = res.results[k]['out'][real]
    return out



# revision 80
# speedup vs baseline: 1.5288x; 1.0002x over previous
"""Trainium2 Bass kernel for nn_DiffusionOrderingNetwork (3-layer GAT, N=50000,
E=800000, softmax over nodes), SPMD across 8 NeuronCores.

Self-contained: host-side index/layout prep + Bass/Tile program + runner.
"""
import sys
sys.path.insert(0, '/opt/trn_rl_repo')
import numpy as np
import ml_dtypes
from contextlib import ExitStack

# ======================= host prep =======================
import numpy as _np

N = 50000
E = 800000
H = 6
C1 = 6
HID = 36
D = 64
NT = 17
NEG = 0.2
NCORES = 8
EPT = 128          # edges per tile
SPT = 8            # node slots per tile
KSUP = 32          # tiles per super-block (layers 1/2) -> 256 psum cols
KSUP3 = 16         # tiles per super-block (layer 3)   -> 128 psum cols


def _fold_ws(W, a):
    # ws[d, h] = sum_c W[d, h*C+c] * a[h, c]
    h, c = a.shape
    return np.einsum('dhc,hc->dh', W.reshape(W.shape[0], h, c), a).astype(np.float32)


def host_prep(x, edge_index, emb, w1, as1, ad1, b1, r1,
              w2, as2, ad2, b2, r2, w3, as3, ad3, b3, r3):
    x = np.asarray(x).astype(np.int64)
    ei = np.asarray(edge_index).astype(np.int64)
    N = x.shape[0]
    NT = emb.shape[0]
    D = emb.shape[1]
    for b in (b1, b2, b3):
        assert np.abs(np.asarray(b)).max() == 0.0, "nonzero bias breaks pad-column math"

    # --- edges sorted by dst; self-loops handled analytically on-device ---
    src = ei[0].copy()
    dst = ei[1].copy()
    order = np.argsort(dst, kind='stable')
    srcs = src[order]
    dsts = dst[order]
    ET = srcs.shape[0]
    deg = np.bincount(dst, minlength=N).astype(np.int64)
    assert deg.max() <= EPT, deg.max()
    node_ptr = np.concatenate([[0], np.cumsum(deg)])  # edge range per node

    # --- shard nodes into NCORES contiguous ranges with ~equal edges ---
    cum = np.cumsum(deg)
    bnds = [0]
    for k in range(1, NCORES):
        bnds.append(int(np.searchsorted(cum, ET * k / NCORES)))
    bnds.append(N)

    # --- per-core straddle tiling: a node's edges may split across two
    # adjacent tiles (never across a 16-tile super boundary); each tile has
    # at most SPT node starts; tiles fill to exactly EPT edges ---
    SB = 16
    core_tiles = []  # per core: list of tiles; tile = list of (node, e0, e1)
    for k in range(NCORES):
        nb, ne = bnds[k], bnds[k + 1]
        tiles = [[]]
        fill, starts = 0, 0
        for n in range(nb, ne):
            d = int(deg[n])
            e0 = int(node_ptr[n])
            if starts == SPT:
                tiles.append([])
                fill, starts = 0, 0
            starts += 1
            if d == 0:
                tiles[-1].append((n, e0, e0))
                continue
            rem = d
            while rem > 0:
                space = EPT - fill
                if space == 0:
                    tiles.append([])
                    fill, starts = 0, 1
                    space = EPT
                take = min(rem, space)
                if take < rem and (len(tiles) % SB) == 0:
                    # straddle would cross a super boundary: start fresh tile
                    tiles.append([])
                    fill, starts = 0, 1
                    take = min(rem, EPT)
                tiles[-1].append((n, e0, e0 + take))
                fill += take
                e0 += take
                rem -= take
        core_tiles.append(tiles)

    lcm = np.lcm(KSUP, np.lcm(KSUP3, 512 // SPT))  # tiles multiple for chunking
    TMAX = max(len(t) for t in core_tiles)
    TMAX = int(-(-TMAX // lcm) * lcm)
    SLOTMAX = TMAX * SPT
    NMAXOUT = max(bnds[k + 1] - bnds[k] for k in range(NCORES))
    NMAXOUT = int(-(-NMAXOUT // 128) * 128)

    # --- global slot-gid map (0 = poison row, then core-major slots);
    # a straddled node's slot lives in the tile of its FIRST edge segment ---
    nodeslot = np.zeros(N, dtype=np.int64)
    node_core = np.zeros(N, dtype=np.int64)
    core_slotof = [dict() for _ in range(NCORES)]  # node -> (tile, slot)
    for k in range(NCORES):
        slotof = core_slotof[k]
        for t, segs in enumerate(core_tiles[k]):
            nstart = 0
            for (n, e0, e1) in segs:
                if n not in slotof:
                    slotof[n] = (t, nstart)
                    nstart += 1
        for n, (t, s) in slotof.items():
            nodeslot[n] = 1 + k * SLOTMAX + t * SPT + s
            node_core[n] = k

    # --- per-core device input arrays ---
    # OHW/OHTW carry a 2-tile slot window per tile: cols/rows [0:SPT) are the
    # PREVIOUS tile's slots (for straddled nodes), [SPT:2*SPT) this tile's.
    W2 = 2 * SPT
    per_core = []
    for k in range(NCORES):
        tiles = core_tiles[k]
        nb, ne = bnds[k], bnds[k + 1]
        nreal = ne - nb
        slotof = core_slotof[k]

        srcgidT = np.zeros((EPT, TMAX), dtype=np.int32)
        OHW = np.zeros((EPT, TMAX * W2), dtype=np.float32)
        OHTW = np.zeros((W2, TMAX * EPT), dtype=np.float32)
        oh17 = np.zeros((NT, SLOTMAX), dtype=np.float32)
        n102 = np.zeros((H * NT, SLOTMAX), dtype=np.float32)
        outrowT = np.zeros((128, SLOTMAX // 128), dtype=np.int32) + 10**7
        slot_node = np.full(SLOTMAX, -1, dtype=np.int64)

        for t, segs in enumerate(tiles):
            row = 0
            for (n, e0, e1) in segs:
                ts, ss = slotof[n]
                assert ts in (t - 1, t)
                wcol = ss + (SPT if ts == t else 0)
                nseg = e1 - e0
                rows = row + np.arange(nseg)
                srcgidT[rows, t] = nodeslot[srcs[e0:e1]]
                OHW[rows, t * W2 + wcol] = 1.0
                OHTW[wcol, t * EPT + rows] = 1.0
                np.add.at(n102, (x[srcs[e0:e1]],
                                 np.full(nseg, ts * SPT + ss)), 1.0)
                row += nseg
                if ts == t:
                    sl = t * SPT + ss
                    oh17[x[n], sl] = 1.0
                    outrowT[sl % 128, sl // 128] = n - nb
                    slot_node[sl] = n - nb
            assert row <= EPT
        # nodes whose slot-tile had no edge segment in that tile (zero-deg or
        # straddle-only) are covered above since slots come from first segs.
        # self-loop counts for the layer-0 histogram:
        own = np.arange(nb, ne)
        sl_own = (nodeslot[own] - 1 - k * SLOTMAX).astype(np.int64)
        np.add.at(n102, (x[own], sl_own), 1.0)
        n102 = np.tile(n102[:NT], (H, 1))

        npadvec = np.full((D, 1), SLOTMAX - nreal, dtype=np.float32)
        per_core.append(dict(
            srcgidT=srcgidT, OH=OHW, OHT=OHTW, oh17=oh17, n102=n102,
            outrowT=outrowT, npadvec=npadvec, nreal=nreal, nb=nb, ne=ne,
            slot_node=slot_node,
        ))

    # --- folded weights (shared across cores) ---
    f32 = np.float32
    Wcat1 = np.concatenate([_fold_ws(w1, as1), w1.astype(f32), _fold_ws(w1, ad1)], axis=1)
    Wcat2 = np.concatenate([_fold_ws(w2, as2), w2.astype(f32), _fold_ws(w2, ad2)], axis=1)
    # layer 3: records carry xin itself (identity block); xs3 scores fold w3/as3
    Wcat3 = np.concatenate([_fold_ws(w3, as3), np.eye(HID, dtype=f32), _fold_ws(w3, ad3)], axis=1)
    # W3stack[h*HID+c, o] = w3[c, h*D+o] / H   (mean over heads folded in)
    W3stack = (w3.reshape(HID, H, D).transpose(1, 0, 2).reshape(H * HID, D) / H).astype(f32)
    REP2 = np.zeros((H, HID), dtype=f32)
    REP2[np.arange(HID) // C1, np.arange(HID)] = 1.0
    d3 = np.arange(H * HID)
    REP3A = np.zeros((H, 128), dtype=f32)
    REP3A[d3[:128] // HID, np.arange(128)] = 1.0
    REP3B = np.zeros((H, H * HID - 128), dtype=f32)
    REP3B[d3[128:] // HID, np.arange(H * HID - 128)] = 1.0
    zrow = np.zeros((SPT, 48), dtype=f32)
    # head selector for the (h, t)-major layer-0 histogram rows
    SELH2 = np.zeros((H * NT, H), dtype=f32)
    SELH2[np.arange(H * NT), np.arange(H * NT) // NT] = 1.0
    # head-replication of hidden features for the layer-3 self-term
    T6A = np.zeros((HID, 128), dtype=f32)
    T6A[np.arange(128) % HID, np.arange(128)] = 1.0
    T6B = np.zeros((HID, H * HID - 128), dtype=f32)
    T6B[np.arange(128, H * HID) % HID, np.arange(H * HID - 128)] = 1.0

    shared = dict(
        emb=emb.astype(f32), embT=emb.astype(f32).T.copy(),
        Wcat1=Wcat1, Wcat2=Wcat2, Wcat3=Wcat3,
        W3stackA=W3stack[:128].copy(), W3stackB=W3stack[128:].copy(),
        r1=r1.astype(f32), r2=r2.astype(f32), r3=r3.astype(f32),
        b1=b1.astype(f32).reshape(-1, 1), b2=b2.astype(f32).reshape(-1, 1),
        b3=b3.astype(f32).reshape(-1, 1),
        REP2=REP2, REP3A=REP3A, REP3B=REP3B, zrow=zrow, SELH2=SELH2,
        T6A=T6A, T6B=T6B,
    )
    meta = dict(TMAX=TMAX, SLOTMAX=SLOTMAX, NMAXOUT=NMAXOUT,
                bnds=bnds, nreal=[pc['nreal'] for pc in per_core],
                slot_node=[pc['slot_node'] for pc in per_core])
    return per_core, shared, meta


def numpy_reference(x, edge_index, emb, w1, as1, ad1, b1, r1,
                    w2, as2, ad2, b2, r2, w3, as3, ad3, b3, r3):
    """Plain numpy port of reference.py for quick host validation."""
    def gat(xf, src, dst, W, a_s, a_d, b, r, concat):
        n = xf.shape[0]
        h, c = a_s.shape
        xs = (xf @ W).reshape(n, h, c)
        a_src = (xs * a_s).sum(-1)
        a_dst = (xs * a_d).sum(-1)
        e = a_src[src] + a_dst[dst]
        e = np.where(e > 0, e, NEG * e)
        m = np.full((n, h), -np.inf)
        np.maximum.at(m, dst, e)
        m = np.where(np.isfinite(m), m, 0.0)
        ex = np.exp(e - m[dst])
        s = np.zeros((n, h))
        np.add.at(s, dst, ex)
        alpha = ex / (s[dst] + 1e-16)
        out = np.zeros((n, h, c))
        np.add.at(out, dst, xs[src] * alpha[:, :, None])
        out = out.reshape(n, h * c) if concat else out.mean(1)
        return out + xf @ r + b

    hf = emb[np.asarray(x).astype(np.int64)]
    loops = np.arange(x.shape[0])
    src = np.concatenate([edge_index[0], loops])
    dst = np.concatenate([edge_index[1], loops])
    hf = np.maximum(gat(hf, src, dst, w1, as1, ad1, b1, r1, True), 0)
    hf = np.maximum(gat(hf, src, dst, w2, as2, ad2, b2, r2, True), 0)
    hf = gat(hf, src, dst, w3, as3, ad3, b3, r3, False)
    hf = hf - hf.max(0, keepdims=True)
    e = np.exp(hf)
    return (e / e.sum(0, keepdims=True)).astype(np.float32)


# ======================= device program =======================

import concourse.bass as bass
import concourse.tile as tile
from concourse import bacc, mybir
from concourse.masks import make_identity
from concourse.tile import add_dep_helper

F32 = mybir.dt.float32
I32 = mybir.dt.int32
BF16 = mybir.dt.bfloat16

H = 6
EPT = 128
SPT = 8
KSUP = 32      # tiles per super for layers 1/2 (256 psum cols)
KSUP3 = 16     # tiles per super for layer 3  (128 psum cols)


def build_program(TMAX, SLOTMAX, NMAXOUT, D, HID, NT, n_cores=8, edge_dt=BF16,
                  debug_dump=False):
    RW = 48                      # record row: asrc(6) | xs(HID=36) | adst(6)
    NCH128 = SLOTMAX // 128
    NCH512 = SLOTMAX // 512
    TROWS = 1 + n_cores * SLOTMAX
    V216 = H * HID               # 216
    VA = 128                     # layer-3 agg split A (dims 0:128)
    VB = V216 - 128              # 88
    cores = list(range(n_cores))

    nc = bacc.Bacc("TRN2", target_bir_lowering=False, debug=False,
                   num_devices=n_cores)

    def din(name, shape, dt=F32):
        return nc.dram_tensor(name, list(shape), dt, kind="ExternalInput")

    W2 = 2 * SPT
    srcg_d = din("srcgidT", [EPT, TMAX], I32)
    oh_d = din("OH", [EPT, TMAX * W2], edge_dt)
    ohtw_d = din("OHTW", [W2, TMAX * EPT], edge_dt)
    oh17_d = din("oh17", [NT, SLOTMAX], edge_dt)
    n102_d = din("n102", [H * NT, SLOTMAX], edge_dt)
    selh2_d = din("SELH2", [H * NT, H], edge_dt)
    t6a_d = din("T6A", [HID, VA], edge_dt)
    t6b_d = din("T6B", [HID, VB], edge_dt)
    outr_d = din("outrowT", [128, NCH128], I32)
    npad_d = din("npadvec", [D, 1])
    embt_d = din("embT", [D, NT], edge_dt)
    wcat_d = [din("Wcat1", [D, RW], edge_dt), din("Wcat2", [HID, RW], edge_dt),
              din("Wcat3", [HID, RW], edge_dt)]
    w3a_d = din("W3stackA", [VA, D], edge_dt)
    w3b_d = din("W3stackB", [VB, D], edge_dt)
    r_d = [din("r1", [D, HID], edge_dt), din("r2", [HID, HID], edge_dt),
           din("r3", [HID, D], edge_dt)]
    b_d = [din("b1", [HID, 1]), din("b2", [HID, 1]), din("b3", [D, 1])]
    rep2_d = din("REP2", [H, HID])
    rep3a_d = din("REP3A", [H, VA])
    rep3b_d = din("REP3B", [H, VB])
    zrow_d = din("zrow", [SPT, RW], edge_dt)
    out_d = nc.dram_tensor("out", [SLOTMAX, D], F32, kind="ExternalOutput")
    if debug_dump:
        dbg1_d = nc.dram_tensor("dbg1", [HID, SLOTMAX], edge_dt, kind="ExternalOutput")
        dbg2_d = nc.dram_tensor("dbg2", [HID, SLOTMAX], edge_dt, kind="ExternalOutput")
        dbg3_d = nc.dram_tensor("dbg3", [D, SLOTMAX], edge_dt, kind="ExternalOutput")

    ag_in = nc.dram_tensor("ag_in", [SLOTMAX, RW], edge_dt)
    table = nc.dram_tensor("table", [TROWS, RW], edge_dt)
    adstL = nc.dram_tensor("adstL", [SPT + SLOTMAX, H], edge_dt)
    cca_i = nc.dram_tensor("cca_i", [D, 1], F32)
    cca_o = nc.dram_tensor("cca_o", [D, 1], F32)
    ccs_i = nc.dram_tensor("ccs_i", [D, 1], F32)
    ccs_o = nc.dram_tensor("ccs_o", [D, 1], F32)

    with ExitStack() as ctx:
        tc = ctx.enter_context(tile.TileContext(nc))
        res = ctx.enter_context(tc.tile_pool(name="res", bufs=1))
        cst = ctx.enter_context(tc.tile_pool(name="cst", bufs=1))
        aux = ctx.enter_context(tc.tile_pool(name="aux", bufs=2, space="PSUM"))
        p1p = ctx.enter_context(tc.tile_pool(name="p1p", bufs=3))

        def load(pool, src, shape, dt=F32, tag=None):
            t = pool.tile(list(shape), dt, tag=tag)
            nc.sync.dma_start(out=t[:], in_=src[:])
            return t

        srcg = res.tile([EPT, TMAX], I32, tag="srcg")
        oht_sb = res.tile([EPT, TMAX * W2], edge_dt, tag="oht")
        outr = load(cst, outr_d, [128, NCH128], I32, tag="outr")
        t6a_sb = load(cst, t6a_d, [HID, VA], edge_dt, tag="t6a")
        t6b_sb = load(cst, t6b_d, [HID, VB], edge_dt, tag="t6b")
        npad_sb = load(cst, npad_d, [D, 1], tag="npad")
        embt_sb = load(cst, embt_d, [D, NT], edge_dt, tag="embt")
        selh2_sb = load(cst, selh2_d, [H * NT, H], edge_dt, tag="selh2")
        wcat_sb = [load(cst, wcat_d[0], [D, RW], edge_dt, tag="wc1"),
                   load(cst, wcat_d[1], [HID, RW], edge_dt, tag="wc2"),
                   load(cst, wcat_d[2], [HID, RW], edge_dt, tag="wc3")]
        w3a_sb = load(cst, w3a_d, [VA, D], edge_dt, tag="w3a")
        w3b_sb = load(cst, w3b_d, [VB, D], edge_dt, tag="w3b")
        r_sb = [load(cst, r_d[0], [D, HID], edge_dt, tag="r1"),
                load(cst, r_d[1], [HID, HID], edge_dt, tag="r2"),
                load(cst, r_d[2], [HID, D], edge_dt, tag="r3")]
        b_sb = [load(cst, b_d[0], [HID, 1], tag="b1"),
                load(cst, b_d[1], [HID, 1], tag="b2"),
                load(cst, b_d[2], [D, 1], tag="b3")]
        rep2_sb = load(cst, rep2_d, [H, HID], tag="rep2")
        rep3a_sb = load(cst, rep3a_d, [H, VA], tag="rep3a")
        rep3b_sb = load(cst, rep3b_d, [H, VB], tag="rep3b")
        idn = cst.tile([64, 64], edge_dt, tag="idn")
        make_identity(nc, idn[:])
        nc.sync.dma_start(out=table[0:1, :], in_=zrow_d[0:1, :])
        adz = nc.sync.dma_start(out=adstL[0:SPT, :], in_=zrow_d[:, 0:H])

        # ---- t17 = per-type layer-1 records [NT, RW]; er1 = emb @ r1 ----
        V102 = H * NT
        t17_sb = cst.tile([NT, RW], edge_dt, tag="t17")
        t17f_sb = cst.tile([NT, RW], F32, tag="t17f")
        er1_sb = cst.tile([NT, HID], edge_dt, tag="er1")
        with tc.tile_pool(name="p17", bufs=1, space="PSUM") as p17:
            ps = p17.tile([NT, RW], F32, space="PSUM", tag="ps")
            nc.tensor.matmul(out=ps[:], lhsT=embt_sb[:], rhs=wcat_sb[0][:],
                             start=True, stop=True)
            nc.vector.tensor_copy(t17_sb[:], ps[:])
            nc.vector.tensor_copy(t17f_sb[:], ps[:])
            pse = p17.tile([NT, HID], F32, space="PSUM", tag="pse")
            nc.tensor.matmul(out=pse[:], lhsT=embt_sb[:], rhs=r_sb[0][:],
                             start=True, stop=True)
            nc.vector.tensor_copy(er1_sb[:], pse[:])

        # ---- layer-0 histogram operands derived from t17 ----
        # rows are (h, t)-major: row h*NT+t
        L_sb = cst.tile([NT, V102], edge_dt, tag="Lsb")       # ad expander
        at102 = cst.tile([V102, 1], F32, tag="at102")         # a_src per (h,t)
        w17t = cst.tile([V102, HID], edge_dt, tag="w17t")     # xs selector
        nc.vector.memset(w17t[:], 0.0)
        for h in range(H):
            nc.vector.tensor_copy(
                L_sb[:, h * NT:(h + 1) * NT],
                t17_sb[:, RW - H + h:RW - H + h + 1].to_broadcast([NT, NT]))
            # partition-shifted moves must go through DMA, not DVE
            nc.sync.dma_start(out=at102[h * NT:(h + 1) * NT, :],
                              in_=t17f_sb[:, h:h + 1])
            c0 = H + h * (HID // H)
            nc.sync.dma_start(
                out=w17t[h * NT:(h + 1) * NT,
                         h * (HID // H):(h + 1) * (HID // H)],
                in_=t17_sb[:, c0:c0 + HID // H])

        lsum = cst.tile([D, 1], F32, tag="lsum")
        lsum2 = cst.tile([D, 1], F32, tag="lsum2")
        # summed (asrc + adst) weight columns for the self-loop terms
        wsum1 = cst.tile([HID, H], edge_dt, tag="wsum1")
        wsum2 = cst.tile([HID, H], edge_dt, tag="wsum2")
        wsum_sb = [wsum1, wsum2]
        for i in (0, 1):
            nc.vector.tensor_tensor(out=wsum_sb[i][:],
                                    in0=wcat_sb[i + 1][:, 0:H],
                                    in1=wcat_sb[i + 1][:, RW - H:RW],
                                    op=mybir.AluOpType.add)

        hT1 = res.tile([HID, SLOTMAX], edge_dt, tag="h36a")
        hT2 = res.tile([HID, SLOTMAX], edge_dt, tag="h36b")
        out3T = res.tile([D, SLOTMAX], edge_dt, tag="h64")
        agg3A = res.tile([VA, SLOTMAX], edge_dt, tag="agg3A")
        agg3B = res.tile([VB, SLOTMAX], edge_dt, tag="agg3B")
        h6a_sb = res.tile([VA, SLOTMAX], edge_dt, tag="h6a")
        h6b_sb = res.tile([VB, SLOTMAX], edge_dt, tag="h6b")

        # pipelined record-phase: emit one 128-col record chunk for layer l
        hmap = {1: hT1, 2: hT2}
        adw_map = {1: [], 2: []}
        agst = {'cc': None}

        def emit_p1(l, c4):
            # one 512-slot group: 4 record matmuls, one sb tile, 2 DMAs
            sb4 = p1p.tile([128, 4 * RW], edge_dt, tag="sb")
            for j in range(4):
                c = 4 * c4 + j
                ps1 = aux.tile([128, 512], F32, space="PSUM", tag="aux")
                nc.tensor.matmul(out=ps1[:, 0:RW],
                                 lhsT=hmap[l][:, c * 128:(c + 1) * 128],
                                 rhs=wcat_sb[l][:], start=True, stop=True)
                nc.scalar.copy(out=sb4[:, j * RW:(j + 1) * RW],
                               in_=ps1[:, 0:RW])
            sb3 = sb4[:].rearrange("p (j d) -> p j d", d=RW)
            wdma = nc.sync.dma_start(
                out=ag_in[c4 * 512:(c4 + 1) * 512,
                          :].rearrange("(j p) d -> p j d", p=128),
                in_=sb3)
            if agst['cc'] is not None:
                for _cc in agst['cc']:
                    add_dep_helper(wdma.ins, _cc.ins,
                                   reason="ag_in WAR vs previous AllGather")
            adw = nc.sync.dma_start(
                out=adstL[SPT + c4 * 512:SPT + (c4 + 1) * 512,
                          :].rearrange("(j p) d -> p j d", p=128),
                in_=sb3[:, :, RW - H:RW])
            adw_map[l].append(adw)

        # ---- layer 0: per-slot type-histogram GAT (no per-edge work) ----
        with tc.tile_pool(name="l0", bufs=3) as p0, \
             tc.tile_pool(name="l0in", bufs=1) as pin, \
             tc.tile_pool(name="l0a", bufs=2, space="PSUM") as pA, \
             tc.tile_pool(name="l0b", bufs=1, space="PSUM") as pB, \
             tc.tile_pool(name="l0c", bufs=1, space="PSUM") as pC:
            oh17_sb = pin.tile([NT, SLOTMAX], edge_dt, tag="oh17s")
            n102_sb = pin.tile([V102, SLOTMAX], edge_dt, tag="n102s")
            nc.sync.dma_start(out=oh17_sb[:, 0:512], in_=oh17_d[:, 0:512])
            nc.sync.dma_start(out=n102_sb[:, 0:512], in_=n102_d[:, 0:512])
            hsm = SLOTMAX // 2
            nc.sync.dma_start(out=oh17_sb[:, 512:hsm], in_=oh17_d[:, 512:hsm])
            nc.sync.dma_start(out=n102_sb[:, 512:hsm], in_=n102_d[:, 512:hsm])
            nc.sync.dma_start(out=oh17_sb[:, hsm:], in_=oh17_d[:, hsm:])
            nc.sync.dma_start(out=n102_sb[:, hsm:], in_=n102_d[:, hsm:])
            for c in range(NCH512):
                csl = slice(c * 512, (c + 1) * 512)
                ohc = oh17_sb[:, csl]
                n102c = n102_sb[:, csl]
                ps102 = pA.tile([V102, 512], F32, space="PSUM", tag="ps102")
                nc.tensor.matmul(out=ps102[:], lhsT=L_sb[:], rhs=ohc,
                                 start=True, stop=True)
                esc = p0.tile([V102, 512], F32, tag="esc0")
                nc.vector.tensor_scalar_add(out=esc[:], in0=ps102[:],
                                            scalar1=at102[:])
                nc.vector.scalar_tensor_tensor(
                    out=esc[:], in0=esc[:], scalar=0.2, in1=esc[:],
                    op0=mybir.AluOpType.mult, op1=mybir.AluOpType.max)
                nc.scalar.activation(out=esc[:], in_=esc[:],
                                     func=mybir.ActivationFunctionType.Exp)
                nE = p0.tile([V102, 512], edge_dt, tag="nE")
                nc.gpsimd.tensor_tensor(out=nE[:], in0=esc[:], in1=n102c,
                                        op=mybir.AluOpType.mult)
                psD = pC.tile([H, 512], F32, space="PSUM", tag="psD")
                nc.tensor.matmul(out=psD[:], lhsT=selh2_sb[:], rhs=nE[:],
                                 start=True, stop=True)
                psN = pB.tile([HID, 512], F32, space="PSUM", tag="psN")
                nc.tensor.matmul(out=psN[:], lhsT=w17t[:], rhs=nE[:],
                                 start=True, stop=True)
                psR = pB.tile([HID, 512], F32, space="PSUM", tag="psR")
                nc.tensor.matmul(out=psR[:], lhsT=er1_sb[:], rhs=ohc,
                                 start=True, stop=True)
                rs = p0.tile([H, 512], F32, tag="rs0")
                nc.vector.tensor_scalar_add(out=rs[:], in0=psD[:],
                                            scalar1=1e-16)
                nc.vector.reciprocal(out=rs[:], in_=rs[:])
                ps2 = pC.tile([HID, 512], F32, space="PSUM", tag="ps20")
                nc.tensor.matmul(out=ps2[:], lhsT=rep2_sb[:], rhs=rs[:],
                                 start=True, stop=True)
                rr = p0.tile([HID, 512], F32, tag="rr0")
                nc.scalar.copy(out=rr[:], in_=ps2[:])
                nc.vector.tensor_tensor(out=hT1[:, csl], in0=psN[:], in1=rr[:],
                                        op=mybir.AluOpType.mult)
                nc.vector.tensor_tensor(out=hT1[:, csl], in0=hT1[:, csl],
                                        in1=psR[:], op=mybir.AluOpType.add)
                nc.scalar.activation(out=hT1[:, csl], in_=hT1[:, csl],
                                     func=mybir.ActivationFunctionType.Relu,
                                     bias=b_sb[0][:])
                emit_p1(1, c)
        nc.sync.dma_start(out=srcg[:], in_=srcg_d[:])
        nc.sync.dma_start(out=oht_sb[:, 0:TMAX * SPT], in_=oh_d[:, 0:TMAX * SPT])
        nc.sync.dma_start(out=oht_sb[:, TMAX * SPT:], in_=oh_d[:, TMAX * SPT:])
        if debug_dump:
            nc.sync.dma_start(out=dbg1_d[:], in_=hT1[:])

        hins = [None, hT1, hT2]
        houts = [None, hT2, None]
        prev_cc = None
        prev_readers = []

        for l in (1, 2):
            hin = hins[l]
            adst_writes = adw_map[l]

            # ---- P2: all-gather the record table (written by pipelined P1) ----
            if n_cores == 1:
                # model the collective as 8 parallel chunk copies
                ccs_l = []
                nch8 = SLOTMAX // 8
                for i8 in range(8):
                    cci = nc.sync.dma_start(
                        out=table[1 + i8 * nch8:1 + (i8 + 1) * nch8, :],
                        in_=ag_in[i8 * nch8:(i8 + 1) * nch8, :])
                    ccs_l.append(cci)
            else:
                ccs_l = [nc.gpsimd.collective_compute(
                    "AllGather", mybir.AluOpType.bypass,
                    replica_groups=[cores],
                    ins=[ag_in[:]], outs=[table[1:, :]],
                )]
            for cc in ccs_l:
                for rd in prev_readers:
                    add_dep_helper(cc.ins, rd.ins,
                                   reason="table WAR vs previous layer gathers")
            prev_cc = ccs_l
            agst['cc'] = ccs_l
            prev_readers = []

            # ---- P3: edge phase (scatter windows are 2 tiles wide: a node
            # may straddle into the next tile; psum accumulates) ----
            ks = KSUP if l < 2 else KSUP3
            nsup = TMAX // ks
            lw = RW - H if l < 2 else H + V216   # scatter lhsT width: 42 / 222
            cols = ks * SPT                      # real psum cols per super
            pcols = cols + SPT                   # + leading ghost window
            with tc.tile_pool(name=f"ed{l}", bufs=3) as wp, \
                 tc.tile_pool(name=f"edp{l}", bufs=1, space="PSUM") as pp, \
                 tc.tile_pool(name=f"eds{l}", bufs=1, space="PSUM") as pps, \
                 tc.tile_pool(name=f"tmp{l}", bufs=1, space="PSUM") as tpp, \
                 tc.tile_pool(name=f"rcp{l}", bufs=1, space="PSUM") as tpr, \
                 tc.tile_pool(name=f"adp{l}", bufs=1, space="PSUM") as adp:
                for g in range(nsup):
                    t0 = g * ks
                    csl = slice(g * cols, (g + 1) * cols)
                    Rg = wp.tile([EPT, ks * RW], edge_dt, tag="Rg")
                    for k in range(ks):
                        gi = nc.gpsimd.indirect_dma_start(
                            out=Rg[:, k * RW:(k + 1) * RW],
                            out_offset=None, in_=table[:],
                            in_offset=bass.IndirectOffsetOnAxis(
                                ap=srcg[:, t0 + k:t0 + k + 1], axis=0))
                        for _cc in prev_cc:
                            add_dep_helper(gi.ins, _cc.ins,
                                           reason="gather RAW AllGather")
                        prev_readers.append(gi)
                    # a_dst expansion operands: 16-row window = prev|own slots
                    ohts = wp.tile([W2, ks * EPT], edge_dt, tag="ohts")
                    nc.sync.dma_start(out=ohts[:],
                                      in_=ohtw_d[:, t0 * EPT:(t0 + ks) * EPT])
                    adsw = wp.tile([W2, ks * H], edge_dt, tag="adsw")
                    adr0 = nc.sync.dma_start(
                        out=adsw[0:SPT, :].rearrange("s (k e) -> s k e", e=H),
                        in_=adstL[t0 * SPT:(t0 + ks) * SPT, :].rearrange(
                            "(k s) e -> s k e", s=SPT))
                    adr1 = nc.sync.dma_start(
                        out=adsw[SPT:W2, :].rearrange("s (k e) -> s k e", e=H),
                        in_=adstL[(t0 + 1) * SPT:(t0 + ks + 1) * SPT,
                                  :].rearrange("(k s) e -> s k e", s=SPT))
                    for c in range(max(0, (t0 * SPT - SPT)) // 512,
                                   ((t0 + ks) * SPT + 511) // 512):
                        add_dep_helper(adr0.ins, adst_writes[c].ins,
                                       reason="ads RAW adstL chunk write")
                        add_dep_helper(adr1.ins, adst_writes[c].ins,
                                       reason="ads RAW adstL chunk write")
                    if g == 0:
                        add_dep_helper(adr0.ins, adz.ins,
                                       reason="ads RAW adstL zero rows")
                    psAD = adp.tile([EPT, ks * H], F32, space="PSUM", tag="psAD")
                    for k in range(ks):
                        nc.tensor.matmul(
                            out=psAD[:, k * H:(k + 1) * H],
                            lhsT=ohts[:, k * EPT:(k + 1) * EPT],
                            rhs=adsw[:, k * H:(k + 1) * H],
                            start=True, stop=True)
                    R3 = Rg[:].rearrange("p (k e) -> p k e", e=RW)
                    esc = wp.tile([EPT, ks * H], F32, tag="esc")
                    nc.vector.tensor_tensor(
                        out=esc[:], in0=R3[:, :, 0:H],
                        in1=psAD[:], op=mybir.AluOpType.add)
                    nc.vector.scalar_tensor_tensor(
                        out=esc[:], in0=esc[:], scalar=0.2, in1=esc[:],
                        op0=mybir.AluOpType.mult, op1=mybir.AluOpType.max)
                    RHS = wp.tile([EPT, ks * lw], edge_dt, tag="RHS")
                    S3 = RHS[:].rearrange("p (k e) -> p k e", e=lw)
                    nc.scalar.activation(
                        out=S3[:, :, 0:H],
                        in_=esc[:].rearrange("p (k h) -> p k h", h=H),
                        func=mybir.ActivationFunctionType.Exp)
                    ex_rep = S3[:, :, 0:H][:, :, :, None].to_broadcast(
                        [EPT, ks, H, lw // H - 1])
                    if l < 2:
                        xs_in = R3[:, :, H:RW - H].rearrange(
                            "p k (h c) -> p k h c", h=H)
                    else:
                        xs_in = R3[:, :, H:RW - H][:, :, None, :].to_broadcast(
                            [EPT, ks, H, HID])
                    nc.vector.tensor_tensor(
                        out=S3[:, :, H:lw].rearrange("p k (h c) -> p k h c", h=H),
                        in0=xs_in, in1=ex_rep, op=mybir.AluOpType.mult)
                    # self-loop term: esc_self = hin.T @ (W_asrc + W_adst)
                    psRec = tpr.tile([H, cols], F32, space="PSUM",
                                     tag="psRec")
                    nc.tensor.matmul(out=psRec[:], lhsT=wsum_sb[l - 1][:],
                                     rhs=hin[:, csl], start=True, stop=True)
                    e3 = wp.tile([H, cols], F32, tag="e3")
                    nc.scalar.copy(out=e3[:], in_=psRec[:])
                    nc.vector.scalar_tensor_tensor(
                        out=e3[:], in0=e3[:], scalar=0.2, in1=e3[:],
                        op0=mybir.AluOpType.mult, op1=mybir.AluOpType.max)
                    nc.scalar.activation(out=e3[:], in_=e3[:],
                                         func=mybir.ActivationFunctionType.Exp)
                    # scatter with 16-col windows, accumulating
                    psS = (pp if l < 2 else pps).tile([H, pcols], F32,
                                                      space="PSUM", tag="psS")
                    nc.vector.memset(psS[:], 0.0)
                    if l < 2:
                        psV = pp.tile([HID, pcols], F32, space="PSUM", tag="psV")
                        nc.vector.memset(psV[:], 0.0)
                    else:
                        psA = pp.tile([VA, pcols], F32, space="PSUM", tag="psA")
                        psB = pp.tile([VB, pcols], F32, space="PSUM", tag="psB")
                        nc.vector.memset(psA[:], 0.0)
                        nc.vector.memset(psB[:], 0.0)
                    for k in range(ks):
                        t = t0 + k
                        ohs = oht_sb[:, t * W2:(t + 1) * W2]
                        wsl = slice(k * SPT, k * SPT + W2)
                        lb = k * lw
                        nc.tensor.matmul(
                            out=psS[:, wsl],
                            lhsT=RHS[:, lb:lb + H], rhs=ohs,
                            start=False, stop=True)
                        if l < 2:
                            nc.tensor.matmul(
                                out=psV[:, wsl],
                                lhsT=RHS[:, lb + H:lb + lw], rhs=ohs,
                                start=False, stop=True)
                        else:
                            nc.tensor.matmul(
                                out=psA[:, wsl],
                                lhsT=RHS[:, lb + H:lb + H + VA], rhs=ohs,
                                start=False, stop=True)
                            nc.tensor.matmul(
                                out=psB[:, wsl],
                                lhsT=RHS[:, lb + H + VA:lb + lw], rhs=ohs,
                                start=False, stop=True)
                    rs = wp.tile([H, cols], F32, tag="rs")
                    nc.vector.scalar_tensor_tensor(
                        out=rs[:], in0=psS[:, SPT:], scalar=1e-16, in1=e3[:],
                        op0=mybir.AluOpType.add, op1=mybir.AluOpType.add)
                    nc.vector.reciprocal(out=rs[:], in_=rs[:])
                    z = wp.tile([H, cols], F32, tag="z")
                    nc.vector.tensor_tensor(out=z[:], in0=e3[:], in1=rs[:],
                                            op=mybir.AluOpType.mult)
                    if l < 2:
                        ps2 = tpp.tile([HID, cols], F32, space="PSUM",
                                       tag="ps2")
                        nc.tensor.matmul(out=ps2[:], lhsT=rep2_sb[:], rhs=rs[:],
                                         start=True, stop=True)
                        rr = wp.tile([HID, cols], F32, tag="rr")
                        nc.scalar.copy(out=rr[:], in_=ps2[:])
                        psZ = tpp.tile([HID, cols], F32, space="PSUM",
                                       tag="ps2")
                        nc.tensor.matmul(out=psZ[:], lhsT=rep2_sb[:], rhs=z[:],
                                         start=True, stop=True)
                        zz = wp.tile([HID, cols], F32, tag="zz")
                        nc.scalar.copy(out=zz[:], in_=psZ[:])
                        # self value xs_self = wcat[:, H:RW-H].T @ hin
                        psXS = tpp.tile([HID, cols], F32, space="PSUM",
                                        tag="ps2")
                        nc.tensor.matmul(out=psXS[:],
                                         lhsT=wcat_sb[l][:, H:RW - H],
                                         rhs=hin[:, csl], start=True, stop=True)
                        nc.vector.tensor_tensor(
                            out=zz[:], in0=zz[:], in1=psXS[:],
                            op=mybir.AluOpType.mult)
                        nc.vector.tensor_tensor(
                            out=rr[:], in0=psV[:, SPT:], in1=rr[:],
                            op=mybir.AluOpType.mult)
                        nc.vector.tensor_tensor(
                            out=houts[l][:, csl], in0=rr[:], in1=zz[:],
                            op=mybir.AluOpType.add)
                    else:
                        for rep_sb, psX, h6, agg, vx in (
                                (rep3a_sb, psA, h6a_sb, agg3A, VA),
                                (rep3b_sb, psB, h6b_sb, agg3B, VB)):
                            ps2X = tpp.tile([vx, cols], F32, space="PSUM",
                                            tag="ps2")
                            nc.tensor.matmul(out=ps2X[:], lhsT=rep_sb[:],
                                             rhs=rs[:], start=True, stop=True)
                            rrX = wp.tile([vx, cols], F32, tag="rrX")
                            nc.scalar.copy(out=rrX[:], in_=ps2X[:])
                            psZX = tpp.tile([vx, cols], F32, space="PSUM",
                                            tag="ps2")
                            nc.tensor.matmul(out=psZX[:], lhsT=rep_sb[:],
                                             rhs=z[:], start=True, stop=True)
                            zzX = wp.tile([vx, cols], F32, tag="zzX")
                            nc.scalar.copy(out=zzX[:], in_=psZX[:])
                            nc.vector.tensor_tensor(
                                out=zzX[:], in0=zzX[:], in1=h6[:, csl],
                                op=mybir.AluOpType.mult)
                            nc.vector.tensor_tensor(
                                out=rrX[:], in0=psX[:, SPT:], in1=rrX[:],
                                op=mybir.AluOpType.mult)
                            nc.vector.tensor_tensor(
                                out=agg[:, csl], in0=rrX[:], in1=zzX[:],
                                op=mybir.AluOpType.add)

                    # ---- pipelined finalize of completed 512-col chunks ----
                    if l == 1 and g % 2 == 1:
                        c5 = g // 2
                        csl5 = slice(c5 * 512, (c5 + 1) * 512)
                        psr = aux.tile([128, 512], F32, space="PSUM", tag="aux")
                        nc.tensor.matmul(out=psr[0:HID, :], lhsT=r_sb[1][:],
                                         rhs=hT1[:, csl5], start=True, stop=True)
                        nc.vector.tensor_tensor(out=hT2[:, csl5],
                                                in0=hT2[:, csl5],
                                                in1=psr[0:HID, :],
                                                op=mybir.AluOpType.add)
                        nc.scalar.activation(
                            out=hT2[:, csl5], in_=hT2[:, csl5],
                            func=mybir.ActivationFunctionType.Relu,
                            bias=b_sb[1][:])
                        emit_p1(2, c5)
                        psh = aux.tile([128, 512], F32, space="PSUM", tag="aux")
                        nc.tensor.matmul(out=psh[0:VA, :], lhsT=t6a_sb[:],
                                         rhs=hT2[:, csl5], start=True, stop=True)
                        nc.scalar.copy(out=h6a_sb[:, csl5], in_=psh[0:VA, :])
                        psh2 = aux.tile([128, 512], F32, space="PSUM", tag="aux")
                        nc.tensor.matmul(out=psh2[0:VB, :], lhsT=t6b_sb[:],
                                         rhs=hT2[:, csl5], start=True, stop=True)
                        nc.scalar.copy(out=h6b_sb[:, csl5], in_=psh2[0:VB, :])
                    if l == 2 and g % 4 == 3:
                        c5 = g // 4
                        csl5 = slice(c5 * 512, (c5 + 1) * 512)
                        ps3 = aux.tile([128, 512], F32, space="PSUM", tag="aux")
                        nc.tensor.matmul(out=ps3[0:D, :], lhsT=w3a_sb[:],
                                         rhs=agg3A[:, csl5],
                                         start=True, stop=False)
                        nc.tensor.matmul(out=ps3[0:D, :], lhsT=w3b_sb[:],
                                         rhs=agg3B[:, csl5],
                                         start=False, stop=False)
                        nc.tensor.matmul(out=ps3[0:D, :], lhsT=r_sb[2][:],
                                         rhs=hT2[:, csl5],
                                         start=False, stop=True)
                        nc.vector.tensor_scalar_add(out=out3T[:, csl5],
                                                    in0=ps3[0:D, :],
                                                    scalar1=b_sb[2][:])
                        # logits are O(1): exp + sum need no max-subtraction
                        nc.scalar.activation(
                            out=out3T[:, csl5], in_=out3T[:, csl5],
                            func=mybir.ActivationFunctionType.Exp)
                        lsc = wp.tile([D, 1], F32, tag="lsc")
                        nc.vector.tensor_reduce(out=lsc[:],
                                                in_=out3T[:, csl5],
                                                axis=mybir.AxisListType.X,
                                                op=mybir.AluOpType.add)
                        acc = lsum if c5 % 2 == 0 else lsum2
                        if c5 < 2:
                            nc.vector.tensor_copy(acc[:], lsc[:])
                        else:
                            nc.vector.tensor_tensor(out=acc[:], in0=acc[:],
                                                    in1=lsc[:],
                                                    op=mybir.AluOpType.add)

            if debug_dump and l == 1:
                nc.sync.dma_start(out=dbg2_d[:], in_=hT2[:])

        # ---- P5: softmax over nodes (global across cores) ----
        with tc.tile_pool(name="sm", bufs=2) as sp, \
             tc.tile_pool(name="smp", bufs=2, space="PSUM") as spp:
            nc.vector.tensor_tensor(out=lsum[:], in0=lsum[:], in1=lsum2[:],
                                    op=mybir.AluOpType.add)
            # pad slots each contribute exp(0) = 1; subtract their count
            nc.vector.tensor_sub(out=lsum[:], in0=lsum[:], in1=npad_sb[:])
            nc.sync.dma_start(out=ccs_i[:], in_=lsum[:])
            if n_cores == 1:
                cc2 = nc.sync.dma_start(out=ccs_o[:], in_=ccs_i[:])
            else:
                cc2 = nc.gpsimd.collective_compute(
                    "AllReduce", mybir.AluOpType.add, replica_groups=[cores],
                    ins=[ccs_i[:]], outs=[ccs_o[:]])
            gsum = sp.tile([D, 1], F32, tag="gsum")
            rb2 = nc.sync.dma_start(out=gsum[:], in_=ccs_o[:])
            add_dep_helper(rb2.ins, cc2.ins, reason="read AllReduce sum result")
            nc.vector.reciprocal(out=gsum[:], in_=gsum[:])
            # fold 1/gsum into a diagonal matmul that also transposes
            diagS = sp.tile([64, 64], edge_dt, tag="diagS")
            nc.vector.tensor_scalar_mul(out=diagS[:], in0=idn[:],
                                        scalar1=gsum[:])
            if debug_dump:
                nc.vector.tensor_scalar_mul(out=out3T[:], in0=out3T[:],
                                            scalar1=gsum[:])
                nc.sync.dma_start(out=dbg3_d[:], in_=out3T[:])
            for c4 in range(NCH128 // 4):
                csl4 = slice(c4 * 512, (c4 + 1) * 512)
                ev4 = sp.tile([128, 4 * D], F32, tag="ev4")
                for j in range(4):
                    trp = spp.tile([128, D], F32, space="PSUM", tag="trp")
                    nc.tensor.matmul(
                        out=trp[:],
                        lhsT=out3T[:, c4 * 512 + j * 128:
                                   c4 * 512 + (j + 1) * 128],
                        rhs=diagS[:], start=True, stop=True)
                    if j % 2 == 0:
                        nc.scalar.copy(out=ev4[:, j * D:(j + 1) * D],
                                       in_=trp[:])
                    else:
                        nc.vector.tensor_copy(ev4[:, j * D:(j + 1) * D],
                                              trp[:])
                nc.sync.dma_start(
                    out=out_d[csl4, :].rearrange("(j p) d -> p j d", p=128),
                    in_=ev4[:].rearrange("p (j d) -> p j d", d=D))

    nc.compile()
    return nc


# ======================= runner =======================
_CACHE = {}


def _make_in_maps(per_core, shared):
    ebf = ml_dtypes.bfloat16
    in_maps = []
    for pc in per_core:
        in_maps.append(dict(
            srcgidT=pc['srcgidT'],
            OH=pc['OH'].astype(ebf),
            OHTW=pc['OHT'].astype(ebf),
            oh17=pc['oh17'].astype(ebf), n102=pc['n102'].astype(ebf),
            SELH2=shared['SELH2'].astype(ebf),
            T6A=shared['T6A'].astype(ebf), T6B=shared['T6B'].astype(ebf),
            outrowT=pc['outrowT'], npadvec=pc['npadvec'],
            embT=shared['embT'].astype(ebf),
            Wcat1=shared['Wcat1'].astype(ebf), Wcat2=shared['Wcat2'].astype(ebf),
            Wcat3=shared['Wcat3'].astype(ebf),
            W3stackA=shared['W3stackA'].astype(ebf),
            W3stackB=shared['W3stackB'].astype(ebf),
            r1=shared['r1'].astype(ebf), r2=shared['r2'].astype(ebf),
            r3=shared['r3'].astype(ebf),
            b1=shared['b1'], b2=shared['b2'], b3=shared['b3'],
            REP2=shared['REP2'], REP3A=shared['REP3A'], REP3B=shared['REP3B'],
            zrow=shared['zrow'].astype(ebf),
        ))
    return in_maps


def kernel(x, edge_index, edge_attr=None, **w):
    """Full inputs in, full [50000, 64] float32 softmax output out."""
    from concourse.bass_utils import run_bass_kernel_spmd
    args = dict(x=x, edge_index=edge_index)
    for k in ('emb', 'w1', 'as1', 'ad1', 'b1', 'r1', 'w2', 'as2', 'ad2', 'b2',
              'r2', 'w3', 'as3', 'ad3', 'b3', 'r3'):
        args[k] = np.asarray(w[k])
    per_core, shared, meta = host_prep(**args)
    key = (meta['TMAX'], meta['SLOTMAX'], meta['NMAXOUT'],
           shared['emb'].shape, shared['r2'].shape)
    if key not in _CACHE:
        _CACHE[key] = build_program(
            meta['TMAX'], meta['SLOTMAX'], meta['NMAXOUT'],
            shared['emb'].shape[1], shared['r2'].shape[0],
            shared['emb'].shape[0])
    nc = _CACHE[key]
    in_maps = _make_in_maps(per_core, shared)
    res = run_bass_kernel_spmd(nc, in_maps, list(range(NCORES)))
    D = shared['emb'].shape[1]
    N = meta['bnds'][-1]
    out = np.zeros((N, D), np.float32)
    for k in range(NCORES):
        nb = meta['bnds'][k]
        sn = meta['slot_node'][k]
        real = sn >= 0
        out[nb + sn[real]] = res.results[k]['out'][real]
    return out



# revision 81
# speedup vs baseline: 1.5339x; 1.0033x over previous
"""Trainium2 Bass kernel for nn_DiffusionOrderingNetwork (3-layer GAT, N=50000,
E=800000, softmax over nodes), SPMD across 8 NeuronCores.

Self-contained: host-side index/layout prep + Bass/Tile program + runner.
"""
import sys
sys.path.insert(0, '/opt/trn_rl_repo')
import numpy as np
import ml_dtypes
from contextlib import ExitStack

# ======================= host prep =======================
import numpy as _np

N = 50000
E = 800000
H = 6
C1 = 6
HID = 36
D = 64
NT = 17
NEG = 0.2
NCORES = 8
EPT = 128          # edges per tile
SPT = 8            # node slots per tile
KSUP = 32          # tiles per super-block (layers 1/2) -> 256 psum cols
KSUP3 = 16         # tiles per super-block (layer 3)   -> 128 psum cols


def _fold_ws(W, a):
    # ws[d, h] = sum_c W[d, h*C+c] * a[h, c]
    h, c = a.shape
    return np.einsum('dhc,hc->dh', W.reshape(W.shape[0], h, c), a).astype(np.float32)


def host_prep(x, edge_index, emb, w1, as1, ad1, b1, r1,
              w2, as2, ad2, b2, r2, w3, as3, ad3, b3, r3):
    x = np.asarray(x).astype(np.int64)
    ei = np.asarray(edge_index).astype(np.int64)
    N = x.shape[0]
    NT = emb.shape[0]
    D = emb.shape[1]
    for b in (b1, b2, b3):
        assert np.abs(np.asarray(b)).max() == 0.0, "nonzero bias breaks pad-column math"

    # --- edges sorted by dst; self-loops handled analytically on-device ---
    src = ei[0].copy()
    dst = ei[1].copy()
    order = np.argsort(dst, kind='stable')
    srcs = src[order]
    dsts = dst[order]
    ET = srcs.shape[0]
    deg = np.bincount(dst, minlength=N).astype(np.int64)
    assert deg.max() <= EPT, deg.max()
    node_ptr = np.concatenate([[0], np.cumsum(deg)])  # edge range per node

    # --- shard nodes into NCORES contiguous ranges with ~equal edges ---
    cum = np.cumsum(deg)
    bnds = [0]
    for k in range(1, NCORES):
        bnds.append(int(np.searchsorted(cum, ET * k / NCORES)))
    bnds.append(N)

    # --- per-core straddle tiling: a node's edges may split across two
    # adjacent tiles (never across a 16-tile super boundary); each tile has
    # at most SPT node starts; tiles fill to exactly EPT edges ---
    SB = 16
    core_tiles = []  # per core: list of tiles; tile = list of (node, e0, e1)
    for k in range(NCORES):
        nb, ne = bnds[k], bnds[k + 1]
        tiles = [[]]
        fill, starts = 0, 0
        for n in range(nb, ne):
            d = int(deg[n])
            e0 = int(node_ptr[n])
            if starts == SPT:
                tiles.append([])
                fill, starts = 0, 0
            starts += 1
            if d == 0:
                tiles[-1].append((n, e0, e0))
                continue
            rem = d
            while rem > 0:
                space = EPT - fill
                if space == 0:
                    tiles.append([])
                    fill, starts = 0, 1
                    space = EPT
                take = min(rem, space)
                if take < rem and (len(tiles) % SB) == 0:
                    # straddle would cross a super boundary: start fresh tile
                    tiles.append([])
                    fill, starts = 0, 1
                    take = min(rem, EPT)
                tiles[-1].append((n, e0, e0 + take))
                fill += take
                e0 += take
                rem -= take
        core_tiles.append(tiles)

    lcm = np.lcm(KSUP, np.lcm(KSUP3, 512 // SPT))  # tiles multiple for chunking
    TMAX = max(len(t) for t in core_tiles)
    TMAX = int(-(-TMAX // lcm) * lcm)
    SLOTMAX = TMAX * SPT
    NMAXOUT = max(bnds[k + 1] - bnds[k] for k in range(NCORES))
    NMAXOUT = int(-(-NMAXOUT // 128) * 128)

    # --- global slot-gid map (0 = poison row, then core-major slots);
    # a straddled node's slot lives in the tile of its FIRST edge segment ---
    nodeslot = np.zeros(N, dtype=np.int64)
    node_core = np.zeros(N, dtype=np.int64)
    core_slotof = [dict() for _ in range(NCORES)]  # node -> (tile, slot)
    for k in range(NCORES):
        slotof = core_slotof[k]
        for t, segs in enumerate(core_tiles[k]):
            nstart = 0
            for (n, e0, e1) in segs:
                if n not in slotof:
                    slotof[n] = (t, nstart)
                    nstart += 1
        for n, (t, s) in slotof.items():
            nodeslot[n] = 1 + k * SLOTMAX + t * SPT + s
            node_core[n] = k

    # --- per-core device input arrays ---
    # OHW/OHTW carry a 2-tile slot window per tile: cols/rows [0:SPT) are the
    # PREVIOUS tile's slots (for straddled nodes), [SPT:2*SPT) this tile's.
    W2 = 2 * SPT
    per_core = []
    for k in range(NCORES):
        tiles = core_tiles[k]
        nb, ne = bnds[k], bnds[k + 1]
        nreal = ne - nb
        slotof = core_slotof[k]

        srcgidT = np.zeros((EPT, TMAX), dtype=np.int32)
        OHW = np.zeros((EPT, TMAX * W2), dtype=np.float32)
        OHTW = np.zeros((W2, TMAX * EPT), dtype=np.float32)
        oh17 = np.zeros((NT, SLOTMAX), dtype=np.float32)
        n102 = np.zeros((H * NT, SLOTMAX), dtype=np.float32)
        outrowT = np.zeros((128, SLOTMAX // 128), dtype=np.int32) + 10**7
        slot_node = np.full(SLOTMAX, -1, dtype=np.int64)

        for t, segs in enumerate(tiles):
            row = 0
            for (n, e0, e1) in segs:
                ts, ss = slotof[n]
                assert ts in (t - 1, t)
                wcol = ss + (SPT if ts == t else 0)
                nseg = e1 - e0
                rows = row + np.arange(nseg)
                srcgidT[rows, t] = nodeslot[srcs[e0:e1]]
                OHW[rows, t * W2 + wcol] = 1.0
                OHTW[wcol, t * EPT + rows] = 1.0
                np.add.at(n102, (x[srcs[e0:e1]],
                                 np.full(nseg, ts * SPT + ss)), 1.0)
                row += nseg
                if ts == t:
                    sl = t * SPT + ss
                    oh17[x[n], sl] = 1.0
                    outrowT[sl % 128, sl // 128] = n - nb
                    slot_node[sl] = n - nb
            assert row <= EPT
        # nodes whose slot-tile had no edge segment in that tile (zero-deg or
        # straddle-only) are covered above since slots come from first segs.
        # self-loop counts for the layer-0 histogram:
        own = np.arange(nb, ne)
        sl_own = (nodeslot[own] - 1 - k * SLOTMAX).astype(np.int64)
        np.add.at(n102, (x[own], sl_own), 1.0)
        n102 = np.tile(n102[:NT], (H, 1))

        npadvec = np.full((D, 1), SLOTMAX - nreal, dtype=np.float32)
        per_core.append(dict(
            srcgidT=srcgidT, OH=OHW, OHT=OHTW, oh17=oh17, n102=n102,
            outrowT=outrowT, npadvec=npadvec, nreal=nreal, nb=nb, ne=ne,
            slot_node=slot_node,
        ))

    # --- folded weights (shared across cores) ---
    f32 = np.float32
    Wcat1 = np.concatenate([_fold_ws(w1, as1), w1.astype(f32), _fold_ws(w1, ad1)], axis=1)
    Wcat2 = np.concatenate([_fold_ws(w2, as2), w2.astype(f32), _fold_ws(w2, ad2)], axis=1)
    # layer 3: records carry xin itself (identity block); xs3 scores fold w3/as3
    Wcat3 = np.concatenate([_fold_ws(w3, as3), np.eye(HID, dtype=f32), _fold_ws(w3, ad3)], axis=1)
    # W3stack[h*HID+c, o] = w3[c, h*D+o] / H   (mean over heads folded in)
    W3stack = (w3.reshape(HID, H, D).transpose(1, 0, 2).reshape(H * HID, D) / H).astype(f32)
    REP2 = np.zeros((H, HID), dtype=f32)
    REP2[np.arange(HID) // C1, np.arange(HID)] = 1.0
    d3 = np.arange(H * HID)
    REP3A = np.zeros((H, 128), dtype=f32)
    REP3A[d3[:128] // HID, np.arange(128)] = 1.0
    REP3B = np.zeros((H, H * HID - 128), dtype=f32)
    REP3B[d3[128:] // HID, np.arange(H * HID - 128)] = 1.0
    zrow = np.zeros((SPT, 48), dtype=f32)
    # head selector for the (h, t)-major layer-0 histogram rows
    SELH2 = np.zeros((H * NT, H), dtype=f32)
    SELH2[np.arange(H * NT), np.arange(H * NT) // NT] = 1.0
    # head-replication of hidden features for the layer-3 self-term
    T6A = np.zeros((HID, 128), dtype=f32)
    T6A[np.arange(128) % HID, np.arange(128)] = 1.0
    T6B = np.zeros((HID, H * HID - 128), dtype=f32)
    T6B[np.arange(128, H * HID) % HID, np.arange(H * HID - 128)] = 1.0

    shared = dict(
        emb=emb.astype(f32), embT=emb.astype(f32).T.copy(),
        Wcat1=Wcat1, Wcat2=Wcat2, Wcat3=Wcat3,
        W3stackA=W3stack[:128].copy(), W3stackB=W3stack[128:].copy(),
        r1=r1.astype(f32), r2=r2.astype(f32), r3=r3.astype(f32),
        b1=b1.astype(f32).reshape(-1, 1), b2=b2.astype(f32).reshape(-1, 1),
        b3=b3.astype(f32).reshape(-1, 1),
        REP2=REP2, REP3A=REP3A, REP3B=REP3B, zrow=zrow, SELH2=SELH2,
        T6A=T6A, T6B=T6B,
    )
    meta = dict(TMAX=TMAX, SLOTMAX=SLOTMAX, NMAXOUT=NMAXOUT,
                bnds=bnds, nreal=[pc['nreal'] for pc in per_core],
                slot_node=[pc['slot_node'] for pc in per_core])
    return per_core, shared, meta


def numpy_reference(x, edge_index, emb, w1, as1, ad1, b1, r1,
                    w2, as2, ad2, b2, r2, w3, as3, ad3, b3, r3):
    """Plain numpy port of reference.py for quick host validation."""
    def gat(xf, src, dst, W, a_s, a_d, b, r, concat):
        n = xf.shape[0]
        h, c = a_s.shape
        xs = (xf @ W).reshape(n, h, c)
        a_src = (xs * a_s).sum(-1)
        a_dst = (xs * a_d).sum(-1)
        e = a_src[src] + a_dst[dst]
        e = np.where(e > 0, e, NEG * e)
        m = np.full((n, h), -np.inf)
        np.maximum.at(m, dst, e)
        m = np.where(np.isfinite(m), m, 0.0)
        ex = np.exp(e - m[dst])
        s = np.zeros((n, h))
        np.add.at(s, dst, ex)
        alpha = ex / (s[dst] + 1e-16)
        out = np.zeros((n, h, c))
        np.add.at(out, dst, xs[src] * alpha[:, :, None])
        out = out.reshape(n, h * c) if concat else out.mean(1)
        return out + xf @ r + b

    hf = emb[np.asarray(x).astype(np.int64)]
    loops = np.arange(x.shape[0])
    src = np.concatenate([edge_index[0], loops])
    dst = np.concatenate([edge_index[1], loops])
    hf = np.maximum(gat(hf, src, dst, w1, as1, ad1, b1, r1, True), 0)
    hf = np.maximum(gat(hf, src, dst, w2, as2, ad2, b2, r2, True), 0)
    hf = gat(hf, src, dst, w3, as3, ad3, b3, r3, False)
    hf = hf - hf.max(0, keepdims=True)
    e = np.exp(hf)
    return (e / e.sum(0, keepdims=True)).astype(np.float32)


# ======================= device program =======================

import concourse.bass as bass
import concourse.tile as tile
from concourse import bacc, mybir
from concourse.masks import make_identity
from concourse.tile import add_dep_helper

F32 = mybir.dt.float32
I32 = mybir.dt.int32
BF16 = mybir.dt.bfloat16

H = 6
EPT = 128
SPT = 8
KSUP = 32      # tiles per super for layers 1/2 (256 psum cols)
KSUP3 = 16     # tiles per super for layer 3  (128 psum cols)


def build_program(TMAX, SLOTMAX, NMAXOUT, D, HID, NT, n_cores=8, edge_dt=BF16,
                  debug_dump=False):
    RW = 48                      # record row: asrc(6) | xs(HID=36) | adst(6)
    NCH128 = SLOTMAX // 128
    NCH512 = SLOTMAX // 512
    TROWS = 1 + n_cores * SLOTMAX
    V216 = H * HID               # 216
    VA = 128                     # layer-3 agg split A (dims 0:128)
    VB = V216 - 128              # 88
    cores = list(range(n_cores))

    nc = bacc.Bacc("TRN2", target_bir_lowering=False, debug=False,
                   num_devices=n_cores)

    def din(name, shape, dt=F32):
        return nc.dram_tensor(name, list(shape), dt, kind="ExternalInput")

    W2 = 2 * SPT
    srcg_d = din("srcgidT", [EPT, TMAX], I32)
    oh_d = din("OH", [EPT, TMAX * W2], edge_dt)
    ohtw_d = din("OHTW", [W2, TMAX * EPT], edge_dt)
    oh17_d = din("oh17", [NT, SLOTMAX], edge_dt)
    n102_d = din("n102", [H * NT, SLOTMAX], edge_dt)
    selh2_d = din("SELH2", [H * NT, H], edge_dt)
    t6a_d = din("T6A", [HID, VA], edge_dt)
    t6b_d = din("T6B", [HID, VB], edge_dt)
    outr_d = din("outrowT", [128, NCH128], I32)
    npad_d = din("npadvec", [D, 1])
    embt_d = din("embT", [D, NT], edge_dt)
    wcat_d = [din("Wcat1", [D, RW], edge_dt), din("Wcat2", [HID, RW], edge_dt),
              din("Wcat3", [HID, RW], edge_dt)]
    w3a_d = din("W3stackA", [VA, D], edge_dt)
    w3b_d = din("W3stackB", [VB, D], edge_dt)
    r_d = [din("r1", [D, HID], edge_dt), din("r2", [HID, HID], edge_dt),
           din("r3", [HID, D], edge_dt)]
    b_d = [din("b1", [HID, 1]), din("b2", [HID, 1]), din("b3", [D, 1])]
    rep2_d = din("REP2", [H, HID])
    rep3a_d = din("REP3A", [H, VA])
    rep3b_d = din("REP3B", [H, VB])
    zrow_d = din("zrow", [SPT, RW], edge_dt)
    out_d = nc.dram_tensor("out", [SLOTMAX, D], F32, kind="ExternalOutput")
    if debug_dump:
        dbg1_d = nc.dram_tensor("dbg1", [HID, SLOTMAX], edge_dt, kind="ExternalOutput")
        dbg2_d = nc.dram_tensor("dbg2", [HID, SLOTMAX], edge_dt, kind="ExternalOutput")
        dbg3_d = nc.dram_tensor("dbg3", [D, SLOTMAX], edge_dt, kind="ExternalOutput")

    ag_in = nc.dram_tensor("ag_in", [SLOTMAX, RW], edge_dt)
    table = nc.dram_tensor("table", [TROWS, RW], edge_dt)
    adstL = nc.dram_tensor("adstL", [SPT + SLOTMAX, H], edge_dt)
    cca_i = nc.dram_tensor("cca_i", [D, 1], F32)
    cca_o = nc.dram_tensor("cca_o", [D, 1], F32)
    ccs_i = nc.dram_tensor("ccs_i", [D, 1], F32)
    ccs_o = nc.dram_tensor("ccs_o", [D, 1], F32)

    with ExitStack() as ctx:
        tc = ctx.enter_context(tile.TileContext(nc))
        res = ctx.enter_context(tc.tile_pool(name="res", bufs=1))
        cst = ctx.enter_context(tc.tile_pool(name="cst", bufs=1))
        aux = ctx.enter_context(tc.tile_pool(name="aux", bufs=2, space="PSUM"))
        p1p = ctx.enter_context(tc.tile_pool(name="p1p", bufs=3))

        def load(pool, src, shape, dt=F32, tag=None):
            t = pool.tile(list(shape), dt, tag=tag)
            nc.sync.dma_start(out=t[:], in_=src[:])
            return t

        srcg = res.tile([EPT, TMAX], I32, tag="srcg")
        oht_sb = res.tile([EPT, TMAX * W2], edge_dt, tag="oht")
        outr = load(cst, outr_d, [128, NCH128], I32, tag="outr")
        t6a_sb = load(cst, t6a_d, [HID, VA], edge_dt, tag="t6a")
        t6b_sb = load(cst, t6b_d, [HID, VB], edge_dt, tag="t6b")
        npad_sb = load(cst, npad_d, [D, 1], tag="npad")
        embt_sb = load(cst, embt_d, [D, NT], edge_dt, tag="embt")
        selh2_sb = load(cst, selh2_d, [H * NT, H], edge_dt, tag="selh2")
        wcat_sb = [load(cst, wcat_d[0], [D, RW], edge_dt, tag="wc1"),
                   load(cst, wcat_d[1], [HID, RW], edge_dt, tag="wc2"),
                   load(cst, wcat_d[2], [HID, RW], edge_dt, tag="wc3")]
        w3a_sb = load(cst, w3a_d, [VA, D], edge_dt, tag="w3a")
        w3b_sb = load(cst, w3b_d, [VB, D], edge_dt, tag="w3b")
        r_sb = [load(cst, r_d[0], [D, HID], edge_dt, tag="r1"),
                load(cst, r_d[1], [HID, HID], edge_dt, tag="r2"),
                load(cst, r_d[2], [HID, D], edge_dt, tag="r3")]
        b_sb = [load(cst, b_d[0], [HID, 1], tag="b1"),
                load(cst, b_d[1], [HID, 1], tag="b2"),
                load(cst, b_d[2], [D, 1], tag="b3")]
        rep2_sb = load(cst, rep2_d, [H, HID], tag="rep2")
        rep3a_sb = load(cst, rep3a_d, [H, VA], tag="rep3a")
        rep3b_sb = load(cst, rep3b_d, [H, VB], tag="rep3b")
        idn = cst.tile([64, 64], edge_dt, tag="idn")
        make_identity(nc, idn[:])
        nc.sync.dma_start(out=table[0:1, :], in_=zrow_d[0:1, :])
        adz = nc.sync.dma_start(out=adstL[0:SPT, :], in_=zrow_d[:, 0:H])

        # ---- t17 = per-type layer-1 records [NT, RW]; er1 = emb @ r1 ----
        V102 = H * NT
        t17_sb = cst.tile([NT, RW], edge_dt, tag="t17")
        t17f_sb = cst.tile([NT, RW], F32, tag="t17f")
        er1_sb = cst.tile([NT, HID], edge_dt, tag="er1")
        with tc.tile_pool(name="p17", bufs=1, space="PSUM") as p17:
            ps = p17.tile([NT, RW], F32, space="PSUM", tag="ps")
            nc.tensor.matmul(out=ps[:], lhsT=embt_sb[:], rhs=wcat_sb[0][:],
                             start=True, stop=True)
            nc.vector.tensor_copy(t17_sb[:], ps[:])
            nc.vector.tensor_copy(t17f_sb[:], ps[:])
            pse = p17.tile([NT, HID], F32, space="PSUM", tag="pse")
            nc.tensor.matmul(out=pse[:], lhsT=embt_sb[:], rhs=r_sb[0][:],
                             start=True, stop=True)
            nc.vector.tensor_copy(er1_sb[:], pse[:])

        # ---- layer-0 histogram operands derived from t17 ----
        # rows are (h, t)-major: row h*NT+t
        L_sb = cst.tile([NT, V102], edge_dt, tag="Lsb")       # ad expander
        at102 = cst.tile([V102, 1], F32, tag="at102")         # a_src per (h,t)
        w17t = cst.tile([V102, HID], edge_dt, tag="w17t")     # xs selector
        nc.vector.memset(w17t[:], 0.0)
        for h in range(H):
            nc.vector.tensor_copy(
                L_sb[:, h * NT:(h + 1) * NT],
                t17_sb[:, RW - H + h:RW - H + h + 1].to_broadcast([NT, NT]))
            # partition-shifted moves must go through DMA, not DVE
            nc.sync.dma_start(out=at102[h * NT:(h + 1) * NT, :],
                              in_=t17f_sb[:, h:h + 1])
            c0 = H + h * (HID // H)
            nc.sync.dma_start(
                out=w17t[h * NT:(h + 1) * NT,
                         h * (HID // H):(h + 1) * (HID // H)],
                in_=t17_sb[:, c0:c0 + HID // H])

        lsum = cst.tile([D, 1], F32, tag="lsum")
        lsum2 = cst.tile([D, 1], F32, tag="lsum2")
        # summed (asrc + adst) weight columns for the self-loop terms
        wsum1 = cst.tile([HID, H], edge_dt, tag="wsum1")
        wsum2 = cst.tile([HID, H], edge_dt, tag="wsum2")
        wsum_sb = [wsum1, wsum2]
        for i in (0, 1):
            nc.vector.tensor_tensor(out=wsum_sb[i][:],
                                    in0=wcat_sb[i + 1][:, 0:H],
                                    in1=wcat_sb[i + 1][:, RW - H:RW],
                                    op=mybir.AluOpType.add)

        hT1 = res.tile([HID, SLOTMAX], edge_dt, tag="h36a")
        hT2 = res.tile([HID, SLOTMAX], edge_dt, tag="h36b")
        out3T = res.tile([D, SLOTMAX], edge_dt, tag="h64")
        agg3A = res.tile([VA, SLOTMAX], edge_dt, tag="agg3A")
        agg3B = res.tile([VB, SLOTMAX], edge_dt, tag="agg3B")
        h6a_sb = res.tile([VA, SLOTMAX], edge_dt, tag="h6a")
        h6b_sb = res.tile([VB, SLOTMAX], edge_dt, tag="h6b")

        # pipelined record-phase: emit one 128-col record chunk for layer l
        hmap = {1: hT1, 2: hT2}
        adw_map = {1: [], 2: []}
        agst = {'cc': None}

        def emit_p1(l, c4):
            # one 512-slot group: 4 record matmuls, one sb tile, 2 DMAs
            sb4 = p1p.tile([128, 4 * RW], edge_dt, tag="sb")
            for j in range(4):
                c = 4 * c4 + j
                ps1 = aux.tile([128, 512], F32, space="PSUM", tag="aux")
                nc.tensor.matmul(out=ps1[:, 0:RW],
                                 lhsT=hmap[l][:, c * 128:(c + 1) * 128],
                                 rhs=wcat_sb[l][:], start=True, stop=True)
                nc.scalar.copy(out=sb4[:, j * RW:(j + 1) * RW],
                               in_=ps1[:, 0:RW])
            sb3 = sb4[:].rearrange("p (j d) -> p j d", d=RW)
            wdma = nc.sync.dma_start(
                out=ag_in[c4 * 512:(c4 + 1) * 512,
                          :].rearrange("(j p) d -> p j d", p=128),
                in_=sb3)
            if agst['cc'] is not None:
                for _cc in agst['cc']:
                    add_dep_helper(wdma.ins, _cc.ins,
                                   reason="ag_in WAR vs previous AllGather")
            adw = nc.sync.dma_start(
                out=adstL[SPT + c4 * 512:SPT + (c4 + 1) * 512,
                          :].rearrange("(j p) d -> p j d", p=128),
                in_=sb3[:, :, RW - H:RW])
            adw_map[l].append(adw)

        # ---- layer 0: per-slot type-histogram GAT (no per-edge work) ----
        with tc.tile_pool(name="l0", bufs=3) as p0, \
             tc.tile_pool(name="l0in", bufs=1) as pin, \
             tc.tile_pool(name="l0a", bufs=2, space="PSUM") as pA, \
             tc.tile_pool(name="l0b", bufs=1, space="PSUM") as pB, \
             tc.tile_pool(name="l0c", bufs=1, space="PSUM") as pC:
            oh17_sb = pin.tile([NT, SLOTMAX], edge_dt, tag="oh17s")
            n102_sb = pin.tile([V102, SLOTMAX], edge_dt, tag="n102s")
            nc.sync.dma_start(out=oh17_sb[:, 0:512], in_=oh17_d[:, 0:512])
            nc.sync.dma_start(out=n102_sb[:, 0:512], in_=n102_d[:, 0:512])
            hsm = SLOTMAX // 2
            nc.sync.dma_start(out=oh17_sb[:, 512:hsm], in_=oh17_d[:, 512:hsm])
            nc.sync.dma_start(out=n102_sb[:, 512:hsm], in_=n102_d[:, 512:hsm])
            nc.sync.dma_start(out=oh17_sb[:, hsm:], in_=oh17_d[:, hsm:])
            nc.sync.dma_start(out=n102_sb[:, hsm:], in_=n102_d[:, hsm:])
            for c in range(NCH512):
                csl = slice(c * 512, (c + 1) * 512)
                ohc = oh17_sb[:, csl]
                n102c = n102_sb[:, csl]
                ps102 = pA.tile([V102, 512], F32, space="PSUM", tag="ps102")
                nc.tensor.matmul(out=ps102[:], lhsT=L_sb[:], rhs=ohc,
                                 start=True, stop=True)
                esc = p0.tile([V102, 512], F32, tag="esc0")
                nc.vector.tensor_scalar_add(out=esc[:], in0=ps102[:],
                                            scalar1=at102[:])
                nc.vector.scalar_tensor_tensor(
                    out=esc[:], in0=esc[:], scalar=0.2, in1=esc[:],
                    op0=mybir.AluOpType.mult, op1=mybir.AluOpType.max)
                nc.scalar.activation(out=esc[:], in_=esc[:],
                                     func=mybir.ActivationFunctionType.Exp)
                nE = p0.tile([V102, 512], edge_dt, tag="nE")
                nc.gpsimd.tensor_tensor(out=nE[:], in0=esc[:], in1=n102c,
                                        op=mybir.AluOpType.mult)
                psD = pC.tile([H, 512], F32, space="PSUM", tag="psD")
                nc.tensor.matmul(out=psD[:], lhsT=selh2_sb[:], rhs=nE[:],
                                 start=True, stop=True)
                psN = pB.tile([HID, 512], F32, space="PSUM", tag="psN")
                nc.tensor.matmul(out=psN[:], lhsT=w17t[:], rhs=nE[:],
                                 start=True, stop=True)
                psR = pB.tile([HID, 512], F32, space="PSUM", tag="psR")
                nc.tensor.matmul(out=psR[:], lhsT=er1_sb[:], rhs=ohc,
                                 start=True, stop=True)
                rs = p0.tile([H, 512], F32, tag="rs0")
                nc.vector.tensor_scalar_add(out=rs[:], in0=psD[:],
                                            scalar1=1e-16)
                nc.vector.reciprocal(out=rs[:], in_=rs[:])
                ps2 = pC.tile([HID, 512], F32, space="PSUM", tag="ps20")
                nc.tensor.matmul(out=ps2[:], lhsT=rep2_sb[:], rhs=rs[:],
                                 start=True, stop=True)
                rr = p0.tile([HID, 512], F32, tag="rr0")
                nc.scalar.copy(out=rr[:], in_=ps2[:])
                nc.vector.tensor_tensor(out=hT1[:, csl], in0=psN[:], in1=rr[:],
                                        op=mybir.AluOpType.mult)
                nc.vector.tensor_tensor(out=hT1[:, csl], in0=hT1[:, csl],
                                        in1=psR[:], op=mybir.AluOpType.add)
                nc.scalar.activation(out=hT1[:, csl], in_=hT1[:, csl],
                                     func=mybir.ActivationFunctionType.Relu,
                                     bias=b_sb[0][:])
                emit_p1(1, c)
        nc.sync.dma_start(out=srcg[:], in_=srcg_d[:])
        nc.sync.dma_start(out=oht_sb[:, 0:TMAX * SPT], in_=oh_d[:, 0:TMAX * SPT])
        nc.sync.dma_start(out=oht_sb[:, TMAX * SPT:], in_=oh_d[:, TMAX * SPT:])
        if debug_dump:
            nc.sync.dma_start(out=dbg1_d[:], in_=hT1[:])

        hins = [None, hT1, hT2]
        houts = [None, hT2, None]
        prev_cc = None
        prev_readers = []

        for l in (1, 2):
            hin = hins[l]
            adst_writes = adw_map[l]

            # ---- P2: all-gather the record table (written by pipelined P1) ----
            if n_cores == 1:
                # model the collective as 8 parallel chunk copies
                ccs_l = []
                nch8 = SLOTMAX // 8
                for i8 in range(8):
                    cci = nc.sync.dma_start(
                        out=table[1 + i8 * nch8:1 + (i8 + 1) * nch8, :],
                        in_=ag_in[i8 * nch8:(i8 + 1) * nch8, :])
                    ccs_l.append(cci)
            else:
                ccs_l = [nc.gpsimd.collective_compute(
                    "AllGather", mybir.AluOpType.bypass,
                    replica_groups=[cores],
                    ins=[ag_in[:]], outs=[table[1:, :]],
                )]
            for cc in ccs_l:
                for rd in prev_readers:
                    add_dep_helper(cc.ins, rd.ins,
                                   reason="table WAR vs previous layer gathers")
            prev_cc = ccs_l
            agst['cc'] = ccs_l
            prev_readers = []

            # ---- P3: edge phase (scatter windows are 2 tiles wide: a node
            # may straddle into the next tile; psum accumulates) ----
            ks = KSUP if l < 2 else KSUP3
            nsup = TMAX // ks
            lw = RW - H if l < 2 else H + V216   # scatter lhsT width: 42 / 222
            cols = ks * SPT                      # real psum cols per super
            pcols = cols + SPT                   # + leading ghost window
            with tc.tile_pool(name=f"ed{l}", bufs=3) as wp, \
                 tc.tile_pool(name=f"edp{l}", bufs=1, space="PSUM") as pp, \
                 tc.tile_pool(name=f"eds{l}", bufs=1, space="PSUM") as pps, \
                 tc.tile_pool(name=f"tmp{l}", bufs=1, space="PSUM") as tpp, \
                 tc.tile_pool(name=f"rcp{l}", bufs=1, space="PSUM") as tpr, \
                 tc.tile_pool(name=f"adp{l}", bufs=1, space="PSUM") as adp:
                for g in range(nsup):
                    t0 = g * ks
                    csl = slice(g * cols, (g + 1) * cols)
                    Rg = wp.tile([EPT, ks * RW], edge_dt, tag="Rg")
                    for k in range(ks):
                        gi = nc.gpsimd.indirect_dma_start(
                            out=Rg[:, k * RW:(k + 1) * RW],
                            out_offset=None, in_=table[:],
                            in_offset=bass.IndirectOffsetOnAxis(
                                ap=srcg[:, t0 + k:t0 + k + 1], axis=0))
                        for _cc in prev_cc:
                            add_dep_helper(gi.ins, _cc.ins,
                                           reason="gather RAW AllGather")
                        prev_readers.append(gi)
                    # a_dst expansion operands: 16-row window = prev|own slots
                    ohts = wp.tile([W2, ks * EPT], edge_dt, tag="ohts")
                    nc.sync.dma_start(out=ohts[:],
                                      in_=ohtw_d[:, t0 * EPT:(t0 + ks) * EPT])
                    adsw = wp.tile([W2, ks * H], edge_dt, tag="adsw")
                    adr0 = nc.sync.dma_start(
                        out=adsw[0:SPT, :].rearrange("s (k e) -> s k e", e=H),
                        in_=adstL[t0 * SPT:(t0 + ks) * SPT, :].rearrange(
                            "(k s) e -> s k e", s=SPT))
                    adr1 = nc.sync.dma_start(
                        out=adsw[SPT:W2, :].rearrange("s (k e) -> s k e", e=H),
                        in_=adstL[(t0 + 1) * SPT:(t0 + ks + 1) * SPT,
                                  :].rearrange("(k s) e -> s k e", s=SPT))
                    for c in range(max(0, (t0 * SPT - SPT)) // 512,
                                   ((t0 + ks) * SPT + 511) // 512):
                        add_dep_helper(adr0.ins, adst_writes[c].ins,
                                       reason="ads RAW adstL chunk write")
                        add_dep_helper(adr1.ins, adst_writes[c].ins,
                                       reason="ads RAW adstL chunk write")
                    if g == 0:
                        add_dep_helper(adr0.ins, adz.ins,
                                       reason="ads RAW adstL zero rows")
                    psAD = adp.tile([EPT, ks * H], F32, space="PSUM", tag="psAD")
                    for k in range(ks):
                        nc.tensor.matmul(
                            out=psAD[:, k * H:(k + 1) * H],
                            lhsT=ohts[:, k * EPT:(k + 1) * EPT],
                            rhs=adsw[:, k * H:(k + 1) * H],
                            start=True, stop=True)
                    R3 = Rg[:].rearrange("p (k e) -> p k e", e=RW)
                    esc = wp.tile([EPT, ks * H], F32, tag="esc")
                    nc.vector.tensor_tensor(
                        out=esc[:], in0=R3[:, :, 0:H],
                        in1=psAD[:], op=mybir.AluOpType.add)
                    nc.vector.scalar_tensor_tensor(
                        out=esc[:], in0=esc[:], scalar=0.2, in1=esc[:],
                        op0=mybir.AluOpType.mult, op1=mybir.AluOpType.max)
                    RHS = wp.tile([EPT, ks * lw], edge_dt, tag="RHS")
                    S3 = RHS[:].rearrange("p (k e) -> p k e", e=lw)
                    nc.scalar.activation(
                        out=S3[:, :, 0:H],
                        in_=esc[:].rearrange("p (k h) -> p k h", h=H),
                        func=mybir.ActivationFunctionType.Exp)
                    ex_rep = S3[:, :, 0:H][:, :, :, None].to_broadcast(
                        [EPT, ks, H, lw // H - 1])
                    if l < 2:
                        xs_in = R3[:, :, H:RW - H].rearrange(
                            "p k (h c) -> p k h c", h=H)
                    else:
                        xs_in = R3[:, :, H:RW - H][:, :, None, :].to_broadcast(
                            [EPT, ks, H, HID])
                    nc.vector.tensor_tensor(
                        out=S3[:, :, H:lw].rearrange("p k (h c) -> p k h c", h=H),
                        in0=xs_in, in1=ex_rep, op=mybir.AluOpType.mult)
                    # self-loop term: esc_self = hin.T @ (W_asrc + W_adst)
                    psRec = tpr.tile([H, cols], F32, space="PSUM",
                                     tag="psRec")
                    nc.tensor.matmul(out=psRec[:], lhsT=wsum_sb[l - 1][:],
                                     rhs=hin[:, csl], start=True, stop=True)
                    e3 = wp.tile([H, cols], F32, tag="e3")
                    nc.scalar.copy(out=e3[:], in_=psRec[:])
                    nc.vector.scalar_tensor_tensor(
                        out=e3[:], in0=e3[:], scalar=0.2, in1=e3[:],
                        op0=mybir.AluOpType.mult, op1=mybir.AluOpType.max)
                    nc.scalar.activation(out=e3[:], in_=e3[:],
                                         func=mybir.ActivationFunctionType.Exp)
                    # scatter with 16-col windows, accumulating
                    psS = (pp if l < 2 else pps).tile([H, pcols], F32,
                                                      space="PSUM", tag="psS")
                    nc.vector.memset(psS[:], 0.0)
                    if l < 2:
                        psV = pp.tile([HID, pcols], F32, space="PSUM", tag="psV")
                        nc.vector.memset(psV[:], 0.0)
                    else:
                        psA = pp.tile([VA, pcols], F32, space="PSUM", tag="psA")
                        psB = pp.tile([VB, pcols], F32, space="PSUM", tag="psB")
                        nc.vector.memset(psA[:], 0.0)
                        nc.vector.memset(psB[:], 0.0)
                    for k in range(ks):
                        t = t0 + k
                        ohs = oht_sb[:, t * W2:(t + 1) * W2]
                        wsl = slice(k * SPT, k * SPT + W2)
                        lb = k * lw
                        nc.tensor.matmul(
                            out=psS[:, wsl],
                            lhsT=RHS[:, lb:lb + H], rhs=ohs,
                            start=False, stop=True)
                        if l < 2:
                            nc.tensor.matmul(
                                out=psV[:, wsl],
                                lhsT=RHS[:, lb + H:lb + lw], rhs=ohs,
                                start=False, stop=True)
                        else:
                            nc.tensor.matmul(
                                out=psA[:, wsl],
                                lhsT=RHS[:, lb + H:lb + H + VA], rhs=ohs,
                                start=False, stop=True)
                            nc.tensor.matmul(
                                out=psB[:, wsl],
                                lhsT=RHS[:, lb + H + VA:lb + lw], rhs=ohs,
                                start=False, stop=True)
                    rs = wp.tile([H, cols], F32, tag="rs")
                    nc.vector.scalar_tensor_tensor(
                        out=rs[:], in0=psS[:, SPT:], scalar=1e-16, in1=e3[:],
                        op0=mybir.AluOpType.add, op1=mybir.AluOpType.add)
                    nc.vector.reciprocal(out=rs[:], in_=rs[:])
                    z = wp.tile([H, cols], F32, tag="z")
                    nc.vector.tensor_tensor(out=z[:], in0=e3[:], in1=rs[:],
                                            op=mybir.AluOpType.mult)
                    if l < 2:
                        ps2 = tpp.tile([HID, cols], F32, space="PSUM",
                                       tag="ps2")
                        nc.tensor.matmul(out=ps2[:], lhsT=rep2_sb[:], rhs=rs[:],
                                         start=True, stop=True)
                        rr = wp.tile([HID, cols], F32, tag="rr")
                        nc.scalar.copy(out=rr[:], in_=ps2[:])
                        psZ = tpp.tile([HID, cols], F32, space="PSUM",
                                       tag="ps2")
                        nc.tensor.matmul(out=psZ[:], lhsT=rep2_sb[:], rhs=z[:],
                                         start=True, stop=True)
                        zz = wp.tile([HID, cols], F32, tag="zz")
                        nc.scalar.copy(out=zz[:], in_=psZ[:])
                        # self value xs_self = wcat[:, H:RW-H].T @ hin
                        psXS = tpp.tile([HID, cols], F32, space="PSUM",
                                        tag="ps2")
                        nc.tensor.matmul(out=psXS[:],
                                         lhsT=wcat_sb[l][:, H:RW - H],
                                         rhs=hin[:, csl], start=True, stop=True)
                        nc.vector.tensor_tensor(
                            out=zz[:], in0=zz[:], in1=psXS[:],
                            op=mybir.AluOpType.mult)
                        nc.vector.tensor_tensor(
                            out=rr[:], in0=psV[:, SPT:], in1=rr[:],
                            op=mybir.AluOpType.mult)
                        nc.vector.tensor_tensor(
                            out=houts[l][:, csl], in0=rr[:], in1=zz[:],
                            op=mybir.AluOpType.add)
                    else:
                        for rep_sb, psX, h6, agg, vx in (
                                (rep3a_sb, psA, h6a_sb, agg3A, VA),
                                (rep3b_sb, psB, h6b_sb, agg3B, VB)):
                            ps2X = tpp.tile([vx, cols], F32, space="PSUM",
                                            tag="ps2")
                            nc.tensor.matmul(out=ps2X[:], lhsT=rep_sb[:],
                                             rhs=rs[:], start=True, stop=True)
                            rrX = wp.tile([vx, cols], F32, tag="rrX")
                            nc.scalar.copy(out=rrX[:], in_=ps2X[:])
                            psZX = tpp.tile([vx, cols], F32, space="PSUM",
                                            tag="ps2")
                            nc.tensor.matmul(out=psZX[:], lhsT=rep_sb[:],
                                             rhs=z[:], start=True, stop=True)
                            zzX = wp.tile([vx, cols], F32, tag="zzX")
                            nc.scalar.copy(out=zzX[:], in_=psZX[:])
                            nc.vector.tensor_tensor(
                                out=zzX[:], in0=zzX[:], in1=h6[:, csl],
                                op=mybir.AluOpType.mult)
                            nc.vector.tensor_tensor(
                                out=rrX[:], in0=psX[:, SPT:], in1=rrX[:],
                                op=mybir.AluOpType.mult)
                            nc.vector.tensor_tensor(
                                out=agg[:, csl], in0=rrX[:], in1=zzX[:],
                                op=mybir.AluOpType.add)

                    # ---- pipelined finalize of completed 512-col chunks ----
                    if l == 1 and g % 2 == 1:
                        c5 = g // 2
                        csl5 = slice(c5 * 512, (c5 + 1) * 512)
                        psr = aux.tile([128, 512], F32, space="PSUM", tag="aux")
                        nc.tensor.matmul(out=psr[0:HID, :], lhsT=r_sb[1][:],
                                         rhs=hT1[:, csl5], start=True, stop=True)
                        nc.vector.tensor_tensor(out=hT2[:, csl5],
                                                in0=hT2[:, csl5],
                                                in1=psr[0:HID, :],
                                                op=mybir.AluOpType.add)
                        nc.scalar.activation(
                            out=hT2[:, csl5], in_=hT2[:, csl5],
                            func=mybir.ActivationFunctionType.Relu,
                            bias=b_sb[1][:])
                        emit_p1(2, c5)
                        psh = aux.tile([128, 512], F32, space="PSUM", tag="aux")
                        nc.tensor.matmul(out=psh[0:VA, :], lhsT=t6a_sb[:],
                                         rhs=hT2[:, csl5], start=True, stop=True)
                        nc.scalar.copy(out=h6a_sb[:, csl5], in_=psh[0:VA, :])
                        psh2 = aux.tile([128, 512], F32, space="PSUM", tag="aux")
                        nc.tensor.matmul(out=psh2[0:VB, :], lhsT=t6b_sb[:],
                                         rhs=hT2[:, csl5], start=True, stop=True)
                        nc.scalar.copy(out=h6b_sb[:, csl5], in_=psh2[0:VB, :])
                    if l == 2 and g % 4 == 3:
                        c5 = g // 4
                        csl5 = slice(c5 * 512, (c5 + 1) * 512)
                        ps3 = aux.tile([128, 512], F32, space="PSUM", tag="aux")
                        nc.tensor.matmul(out=ps3[0:D, :], lhsT=w3a_sb[:],
                                         rhs=agg3A[:, csl5],
                                         start=True, stop=False)
                        nc.tensor.matmul(out=ps3[0:D, :], lhsT=w3b_sb[:],
                                         rhs=agg3B[:, csl5],
                                         start=False, stop=False)
                        nc.tensor.matmul(out=ps3[0:D, :], lhsT=r_sb[2][:],
                                         rhs=hT2[:, csl5],
                                         start=False, stop=True)
                        nc.vector.tensor_scalar_add(out=out3T[:, csl5],
                                                    in0=ps3[0:D, :],
                                                    scalar1=b_sb[2][:])
                        # logits are O(1): exp + sum need no max-subtraction
                        nc.scalar.activation(
                            out=out3T[:, csl5], in_=out3T[:, csl5],
                            func=mybir.ActivationFunctionType.Exp)
                        lsc = wp.tile([D, 1], F32, tag="lsc")
                        nc.vector.tensor_reduce(out=lsc[:],
                                                in_=out3T[:, csl5],
                                                axis=mybir.AxisListType.X,
                                                op=mybir.AluOpType.add)
                        acc = lsum if c5 % 2 == 0 else lsum2
                        if c5 < 2:
                            nc.vector.tensor_copy(acc[:], lsc[:])
                        else:
                            nc.vector.tensor_tensor(out=acc[:], in0=acc[:],
                                                    in1=lsc[:],
                                                    op=mybir.AluOpType.add)

            if debug_dump and l == 1:
                nc.sync.dma_start(out=dbg2_d[:], in_=hT2[:])

        # ---- P5: softmax over nodes (global across cores) ----
        with tc.tile_pool(name="sm", bufs=2) as sp, \
             tc.tile_pool(name="smp", bufs=2, space="PSUM") as spp:
            nc.vector.tensor_tensor(out=lsum[:], in0=lsum[:], in1=lsum2[:],
                                    op=mybir.AluOpType.add)
            # pad slots each contribute exp(0) = 1; subtract their count
            nc.vector.tensor_sub(out=lsum[:], in0=lsum[:], in1=npad_sb[:])
            nc.sync.dma_start(out=ccs_i[:], in_=lsum[:])
            if n_cores == 1:
                cc2 = nc.sync.dma_start(out=ccs_o[:], in_=ccs_i[:])
            else:
                cc2 = nc.gpsimd.collective_compute(
                    "AllReduce", mybir.AluOpType.add, replica_groups=[cores],
                    ins=[ccs_i[:]], outs=[ccs_o[:]])
            gsum = sp.tile([D, 1], F32, tag="gsum")
            rb2 = nc.sync.dma_start(out=gsum[:], in_=ccs_o[:])
            add_dep_helper(rb2.ins, cc2.ins, reason="read AllReduce sum result")
            nc.vector.reciprocal(out=gsum[:], in_=gsum[:])
            # fold 1/gsum into a diagonal matmul that also transposes
            diagS = sp.tile([64, 64], edge_dt, tag="diagS")
            nc.vector.tensor_scalar_mul(out=diagS[:], in0=idn[:],
                                        scalar1=gsum[:])
            if debug_dump:
                nc.vector.tensor_scalar_mul(out=out3T[:], in0=out3T[:],
                                            scalar1=gsum[:])
                nc.sync.dma_start(out=dbg3_d[:], in_=out3T[:])
            g0 = 0
            while g0 < NCH128:
                gw = min(8, NCH128 - g0)
                ev4 = sp.tile([128, 8 * D], F32, tag="ev4")
                for j in range(gw):
                    trp = spp.tile([128, D], F32, space="PSUM", tag="trp")
                    nc.tensor.matmul(
                        out=trp[:],
                        lhsT=out3T[:, (g0 + j) * 128:(g0 + j + 1) * 128],
                        rhs=diagS[:], start=True, stop=True)
                    if j % 2 == 0:
                        nc.scalar.copy(out=ev4[:, j * D:(j + 1) * D],
                                       in_=trp[:])
                    else:
                        nc.vector.tensor_copy(ev4[:, j * D:(j + 1) * D],
                                              trp[:])
                nc.sync.dma_start(
                    out=out_d[g0 * 128:(g0 + gw) * 128,
                              :].rearrange("(j p) d -> p j d", p=128),
                    in_=ev4[:, 0:gw * D].rearrange("p (j d) -> p j d", d=D))
                g0 += gw

    nc.compile()
    return nc


# ======================= runner =======================
_CACHE = {}


def _make_in_maps(per_core, shared):
    ebf = ml_dtypes.bfloat16
    in_maps = []
    for pc in per_core:
        in_maps.append(dict(
            srcgidT=pc['srcgidT'],
            OH=pc['OH'].astype(ebf),
            OHTW=pc['OHT'].astype(ebf),
            oh17=pc['oh17'].astype(ebf), n102=pc['n102'].astype(ebf),
            SELH2=shared['SELH2'].astype(ebf),
            T6A=shared['T6A'].astype(ebf), T6B=shared['T6B'].astype(ebf),
            outrowT=pc['outrowT'], npadvec=pc['npadvec'],
            embT=shared['embT'].astype(ebf),
            Wcat1=shared['Wcat1'].astype(ebf), Wcat2=shared['Wcat2'].astype(ebf),
            Wcat3=shared['Wcat3'].astype(ebf),
            W3stackA=shared['W3stackA'].astype(ebf),
            W3stackB=shared['W3stackB'].astype(ebf),
            r1=shared['r1'].astype(ebf), r2=shared['r2'].astype(ebf),
            r3=shared['r3'].astype(ebf),
            b1=shared['b1'], b2=shared['b2'], b3=shared['b3'],
            REP2=shared['REP2'], REP3A=shared['REP3A'], REP3B=shared['REP3B'],
            zrow=shared['zrow'].astype(ebf),
        ))
    return in_maps


def kernel(x, edge_index, edge_attr=None, **w):
    """Full inputs in, full [50000, 64] float32 softmax output out."""
    from concourse.bass_utils import run_bass_kernel_spmd
    args = dict(x=x, edge_index=edge_index)
    for k in ('emb', 'w1', 'as1', 'ad1', 'b1', 'r1', 'w2', 'as2', 'ad2', 'b2',
              'r2', 'w3', 'as3', 'ad3', 'b3', 'r3'):
        args[k] = np.asarray(w[k])
    per_core, shared, meta = host_prep(**args)
    key = (meta['TMAX'], meta['SLOTMAX'], meta['NMAXOUT'],
           shared['emb'].shape, shared['r2'].shape)
    if key not in _CACHE:
        _CACHE[key] = build_program(
            meta['TMAX'], meta['SLOTMAX'], meta['NMAXOUT'],
            shared['emb'].shape[1], shared['r2'].shape[0],
            shared['emb'].shape[0])
    nc = _CACHE[key]
    in_maps = _make_in_maps(per_core, shared)
    res = run_bass_kernel_spmd(nc, in_maps, list(range(NCORES)))
    D = shared['emb'].shape[1]
    N = meta['bnds'][-1]
    out = np.zeros((N, D), np.float32)
    for k in range(NCORES):
        nb = meta['bnds'][k]
        sn = meta['slot_node'][k]
        real = sn >= 0
        out[nb + sn[real]] = res.results[k]['out'][real]
    return out

